# revision 49
# baseline (speedup 1.0000x reference)
"""Trainium2 Bass kernel for the DTI R-GCN (bdd) model, 8 NeuronCores.

v2 strategy (SPMD, one program, per-core data):
  - dst-shard the graph: core c owns nodes [c*2500, (c+1)*2500). Edges are
    bucketed by (dst-tile-of-256, rel-pair, rel, dst-quarter-of-64); bucket
    sizes are padded to the max over the 8 cores so offsets are compile-time
    and shared (pad slots carry norm=0 so they contribute nothing).
  - the padded edge stream is gathered in contiguous 128-edge windows,
    G windows per indirect-DMA call (amortizes the ~1us SWDGE fixed cost),
    from a bf16 copy of the node features.
  - scatter: per 128-edge window one S matrix [128, 64] per bucket-overlap
    (S[e, d] = norm_e * (iota64 == dq_e), rows outside the bucket zeroed via
    norm=0), and one matmul per (overlap, feature-half) accumulates
    aggT[fi, rel-pair 512] in a full PSUM bank -- free dim 64 so scatter PE
    cost is ~4x lower than 256-wide chunks.
  - per (dst-tile, rel-pair): one [128, 512] PSUM->SBUF copy per fi-half,
    then 2 matmuls per rel apply the block-diagonal W (free dim 256, f32r).
    Rel-apply runs one group late to hide the copy latency.
  - self-loop reads resident transposed features (x0T uploaded by host for
    layer 1; h1T written into SBUF during layer 1 with bias folded into the
    PSUM->SBUF activation copy), so no per-tile loads or transposes.
  - layer outputs AllGather piecewise directly INTO the next layer's
    gather-table layout (piece-major rows), so no DRAM fold pass; gather
    indices are host-remapped to that layout.
  - MLP head is data-parallel over pairs (512 per core, free-dim-512 f32r).
"""
import sys

sys.path.insert(0, "/opt/trn_rl_repo")
import numpy as np
import ml_dtypes

P = 128
QN = 64          # dst quarter width (scatter matmul free dim)
T2SZ = 256       # dst nodes per msgT tile / rel-apply free dim
NCORES = 8
G = 8            # windows per gather call
NXGW = 8         # gather tile bufs
NSBUF = 28       # S tile bufs


def _round_f32r(x):
    u = np.ascontiguousarray(x, np.float32).view(np.uint32)
    u = (u + 0x7FF + ((u >> 12) & 1)) & np.uint32(0xFFFFF000)
    return u.view(np.float32)


def _wrap16(flat):
    assert len(flat) % 16 == 0
    w = np.asarray(flat, np.int16).reshape(-1, 16).T.copy()
    return np.tile(w, (8, 1))


def _preprocess(inputs, ncores=NCORES):
    node_ids = np.asarray(inputs["node_ids"])
    src = np.asarray(inputs["src"])
    dst = np.asarray(inputs["dst"])
    etype = np.asarray(inputs["etype"])
    norm = np.asarray(inputs["norm"]).reshape(-1)
    emb = np.asarray(inputs["emb"], dtype=np.float32)
    drugs = np.asarray(inputs["drugs_index"])
    targets = np.asarray(inputs["targets_index"])

    N = node_ids.shape[0]
    H = emb.shape[1]
    R = int(inputs["w1"].shape[0])
    RP = R // 2
    PAIRS = drugs.shape[0]
    assert N % ncores == 0 and PAIRS % ncores == 0
    NOWN = N // ncores
    T2 = -(-NOWN // T2SZ)
    PPC = PAIRS // ncores
    Q = PPC // P
    NBUK = T2 * R * 4

    # ---- node-position balancing ----
    # We are free to choose which owned node occupies which position in the
    # core's [0, NOWN) range (positions define dst tiles/quarters, the
    # output layout, and the gather-table rows -- all remapped consistently
    # below). Greedily assign nodes to (t2, quarter) bins so each bin's
    # per-relation in-degree is as uniform as possible: bucket sizes are
    # padded to the max over cores, so flattening per-core bucket counts
    # directly shrinks the padded edge stream (gather bytes + PE area).
    NBIN = T2 * 4
    owner = dst // NOWN
    pos_g = np.zeros(N, np.int64)  # node -> position within its core
    cap0 = np.full(NBIN, QN, np.int64)
    tail = NOWN - (T2 - 1) * T2SZ  # rows in the last t2
    for q in range(4):
        cap0[(T2 - 1) * 4 + q] = min(QN, max(0, tail - q * QN))
    for c in range(ncores):
        m = owner == c
        ind = np.zeros((NOWN, R), np.float64)
        np.add.at(ind, (dst[m] - c * NOWN, etype[m]), 1.0)
        tot = ind.sum(axis=0)
        cap = cap0.copy()
        target = np.outer(cap0 / float(NOWN), tot)  # [NBIN, R]
        load = np.zeros((NBIN, R), np.float64)
        order = np.argsort(-ind.sum(axis=1), kind="stable")
        nexti = np.zeros(NBIN, np.int64)
        for n in order:
            score = (load - target) @ ind[n]
            score = np.where(cap > 0, score, np.inf)
            b = int(np.argmin(score))
            load[b] += ind[n]
            cap[b] -= 1
            t2b, qb = b // 4, b % 4
            pos_g[c * NOWN + n] = t2b * T2SZ + qb * QN + nexti[b]
            nexti[b] += 1

    # ---- edge bucketing: b = ((t2*RP + rp)*2 + s)*4 + q ----
    d = pos_g[dst]
    t2_e = d // T2SZ
    q_e = (d % T2SZ) // QN
    dq_e = (d % QN).astype(np.float32)
    rp_e = etype // 2
    s_e = etype % 2
    b_e = ((t2_e * RP + rp_e) * 2 + s_e) * 4 + q_e

    counts = np.zeros((ncores, NBUK), np.int64)
    for c in range(ncores):
        counts[c] = np.bincount(b_e[owner == c], minlength=NBUK)
    sz = np.maximum(counts.max(axis=0), 1)
    off = np.zeros(NBUK, np.int64)
    off[1:] = np.cumsum(sz)[:-1]
    TE = int(off[-1] + sz[-1])
    NW = -(-TE // P)
    TEp = NW * P

    # per-core padded slot arrays
    slot_src = np.zeros((ncores, TEp), np.int32)
    slot_dq = np.zeros((ncores, TEp), np.float32)
    slot_norm = np.zeros((ncores, TEp), np.float32)
    for c in range(ncores):
        m = owner == c
        eidx = np.where(m)[0]
        bb = b_e[eidx]
        order = np.argsort(bb, kind="stable")
        eidx = eidx[order]
        bb = bb[order]
        cstart = np.zeros(NBUK, np.int64)
        cstart[1:] = np.cumsum(counts[c])[:-1]
        rank = np.arange(len(eidx)) - cstart[bb]
        pos = off[bb] + rank
        slot_src[c, pos] = src[eidx]
        slot_dq[c, pos] = dq_e[eidx]
        slot_norm[c, pos] = norm[eidx]

    # ---- overlap enumeration at (window x group) granularity ----
    # Buckets of a (t2, rel-pair) group that fall in the same 128-edge
    # window share ONE S matrix: S columns are bucket-relative
    # (dq' = (bucket_pos - first_bucket)*64 + dq), so each edge row's
    # single nonzero lands in its own bucket's 64-column range -- no
    # masking between buckets of the group is needed. Rows outside the
    # group (window crossing a group boundary) are masked via norm=0.
    # Each overlap becomes 1-2 matmuls: a continuation part (the overlap's
    # first bucket continuing from the previous window; start=False) and a
    # fresh part (buckets starting in this window; start=True).
    # groups[t2*RP+rp] = list of overlap dicts
    NGRP = T2 * RP
    groups = [[] for _ in range(NGRP)]
    ov_info = []  # (w, glo, ghi) rows of window belonging to this overlap
    SWMAX = 0
    for g in range(NGRP):
        b0, b1 = g * 8, g * 8 + 8  # bucket range of group
        glo, ghi = int(off[b0]), int(off[b1 - 1] + sz[b1 - 1])
        for w in range(glo // P, (ghi - 1) // P + 1):
            wlo, whi = max(glo, w * P), min(ghi, (w + 1) * P)
            # buckets intersecting [wlo, whi)
            bs = [
                b for b in range(b0, b1)
                if off[b] < whi and off[b] + sz[b] > wlo
            ]
            fb = bs[0] - b0  # first bucket pos in group (0..7)
            span = (bs[-1] - bs[0] + 1) * QN
            SWMAX = max(SWMAX, span)
            k = len(ov_info)
            ov_info.append((w, wlo, whi))
            cont = off[bs[0]] < wlo  # first bucket started earlier
            parts = []
            if cont:
                stop0 = (off[bs[0]] + sz[bs[0]]) <= whi
                parts.append((0, QN, False, stop0))  # S cols, start, stop
                if len(bs) > 1:
                    stop1 = (off[bs[-1]] + sz[bs[-1]]) <= whi
                    parts.append((QN, span, True, stop1))
            else:
                stop1 = (off[bs[-1]] + sz[bs[-1]]) <= whi
                parts.append((0, span, True, stop1))
            groups[g].append(dict(w=w, k=k, fb=fb, span=span, parts=parts))
    NOV = len(ov_info)
    # iota/dq' compare runs in bf16, exact only for integers <= 256
    assert SWMAX <= 256, f"S span {SWMAX} exceeds bf16-exact range"

    # per-core overlap columns: dq' with bucket-relative column offset,
    # norm masked to the group's rows
    dqT = np.zeros((ncores, P, NOV), np.float32)
    normT = np.zeros((ncores, P, NOV), np.float32)
    # per-slot bucket pos within its group (0..7), from offsets
    slot_bpos = np.zeros(TEp, np.int64)
    for b in range(NBUK):
        slot_bpos[off[b] : off[b] + sz[b]] = b % 8
    rows = np.arange(P)
    for g in range(NGRP):
        for ov in groups[g]:
            w, k = ov["w"], ov["k"]
            _, wlo, whi = ov_info[k]
            sl = slice(w * P, (w + 1) * P)
            mask = (rows >= (wlo - w * P)) & (rows < (whi - w * P))
            first_b = ov["fb"]
            rel = (slot_bpos[sl] - first_b) * QN
            for c in range(ncores):
                dqT[c, :, k] = (slot_dq[c, sl] + rel) * mask
                normT[c, :, k] = slot_norm[c, sl] * mask

    # ---- allgather piece structure: pieces of 3 dst-tiles (t2 groups) ----
    # piece p covers t2 [3p, min(3p+3, T2)); rows per t2 = 256 (last: rem)
    pieces_t2 = []
    t20 = 0
    while t20 < T2:
        t21 = min(t20 + 3, T2)
        r0 = t20 * T2SZ
        r1 = min(NOWN, t21 * T2SZ)
        pieces_t2.append((t20, t21, r1 - r0))
        t20 = t21
    NP_ = len(pieces_t2)
    psizes = [pz for (_, _, pz) in pieces_t2]
    pbase = np.zeros(NP_ + 1, np.int64)
    pbase[1:] = np.cumsum([ncores * s for s in psizes])
    NTAB = int(pbase[-1])

    # node id -> piece-layout row in the allgathered table
    def piecemap(n):
        n = np.asarray(n, np.int64)
        c2 = n // NOWN
        rr = pos_g[n]
        t2i = rr // T2SZ
        p = np.minimum(t2i // 3, NP_ - 1)
        szp = np.asarray(psizes, np.int64)[p]
        start = np.asarray([a * 3 * T2SZ for a in range(NP_)], np.int64)[p]
        return (pbase[p] + c2 * szp + (rr - start)).astype(np.int32)

    # gather index uploads
    srcW1 = np.stack([_wrap16(slot_src[c]) for c in range(ncores)])
    src2 = piecemap(slot_src)  # [ncores, TEp]
    srcW2 = np.stack([_wrap16(src2[c]) for c in range(ncores)])
    # head pair reorder: pairs whose drug/target row falls in the last
    # allgather piece go in the final 128 slots, so the first 384 pairs can
    # gather from the pieces-1..3 table slice while the last piece computes
    pbase3 = int(pbase[NP_ - 1])
    head_perm = np.zeros((ncores, PPC), np.int64)
    drows = np.zeros((ncores, PPC), np.int32)
    trows = np.zeros((ncores, PPC), np.int32)
    for c in range(ncores):
        dr = piecemap(drugs[c * PPC : (c + 1) * PPC])
        tr = piecemap(targets[c * PPC : (c + 1) * PPC])
        late = (dr >= pbase3) | (tr >= pbase3)
        perm = np.argsort(late, kind="stable")
        head_perm[c] = perm
        drows[c] = dr[perm]
        trows[c] = tr[perm]
    nlate = np.array(
        [((drows[c] >= pbase3) | (trows[c] >= pbase3)).sum() for c in range(ncores)]
    )
    HSPLIT = (3 if nlate.max() <= P else 2) * P  # stage-A pair count
    drugsW = np.stack([_wrap16(drows[c]) for c in range(ncores)])
    targetsW = np.stack([_wrap16(trows[c]) for c in range(ncores)])

    # ---- features ----
    h0 = emb[node_ids]  # [N, H]
    h0b = h0.astype(ml_dtypes.bfloat16)
    # resident transposed own features: x0T[p, h*(T2*T2SZ) + t2*T2SZ + dd]
    x0T = np.zeros((ncores, P, 2 * T2 * T2SZ), np.float32)
    for c in range(ncores):
        pad = np.zeros((T2 * T2SZ, H), np.float32)
        pad[pos_g[c * NOWN : (c + 1) * NOWN]] = h0[c * NOWN : (c + 1) * NOWN]
        x0T[c] = _round_f32r(
            pad.reshape(T2 * T2SZ, 2, P).transpose(2, 1, 0).reshape(P, 2 * T2 * T2SZ)
        )

    # ---- weights ----
    B = int(inputs["w1"].shape[1])
    si = H // B
    hb = P // si
    wblk = np.zeros((2, R, 2, P, P), np.float32)
    for l, W in enumerate([inputs["w1"], inputs["w2"]]):
        W = np.asarray(W, np.float32)
        for r in range(R):
            for hh in range(2):
                for bb in range(hb):
                    bidx = hb * hh + bb
                    wblk[l, r, hh, bb * si : (bb + 1) * si, bb * si : (bb + 1) * si] = (
                        W[r, bidx]
                    )
    wblk_in = _round_f32r(wblk.transpose(3, 0, 1, 2, 4).reshape(P, 2 * R * 2 * P))
    loopw = np.stack(
        [np.asarray(inputs["loop_w1"], np.float32), np.asarray(inputs["loop_w2"], np.float32)]
    )
    loopw_in = _round_f32r(
        loopw.reshape(2, 2, P, H).transpose(2, 0, 1, 3).reshape(P, 2 * 2 * H)
    )
    biasT = np.zeros((P, 4), np.float32)  # col l*2 + ho
    for l, bkey in enumerate(["b1", "b2"]):
        bv = np.asarray(inputs[bkey], np.float32)
        for ho in range(2):
            biasT[:, l * 2 + ho] = bv[ho * P : (ho + 1) * P]

    d2 = 2 * H
    KC = d2 // P
    MC = d2 // P
    fc1_in = _round_f32r(
        np.asarray(inputs["fc1_W"], np.float32)
        .reshape(KC, P, MC, P)
        .transpose(1, 0, 2, 3)
        .reshape(P, KC * MC * P)
    )
    fc1b_in = np.asarray(inputs["fc1_b"], np.float32).reshape(MC, P).T.copy()
    fc2_in = _round_f32r(np.asarray(inputs["fc2_W"], np.float32).reshape(MC, P).T)
    fc2b = float(np.asarray(inputs["fc2_b"]).reshape(-1)[0])

    iota64 = np.tile(np.arange(SWMAX, dtype=ml_dtypes.bfloat16), (P, 1))

    meta = dict(
        N=N, H=H, R=R, RP=RP, NOWN=NOWN, T2=T2, NW=NW, NOV=NOV, Q=Q,
        KC=KC, MC=MC, fc2b=fc2b, groups=groups, pieces_t2=pieces_t2,
        psizes=psizes, pbase=pbase, NTAB=NTAB, SWMAX=SWMAX,
        HSPLIT=HSPLIT, head_perm=head_perm, pbase3=pbase3,
    )
    shared = dict(
        h0b=h0b, iota64=iota64, wblk=wblk_in, loopw=loopw_in, biasT=biasT,
        fc1=fc1_in, fc1b=fc1b_in, fc2=fc2_in,
    )
    in_maps = []
    for c in range(ncores):
        m = dict(shared)
        m.update(
            srcW1=srcW1[c], srcW2=srcW2[c], dqT=dqT[c], normT=normT[c],
            x0T=x0T[c], drugsW=drugsW[c], targetsW=targetsW[c],
        )
        in_maps.append(m)
    return meta, in_maps


def _build(meta, ncores=NCORES, single=False):
    from concourse import bass, mybir, tile, bacc
    from concourse.masks import make_identity

    N, H, R, RP = meta["N"], meta["H"], meta["R"], meta["RP"]
    NOWN, T2, NW, NOV, Q = meta["NOWN"], meta["T2"], meta["NW"], meta["NOV"], meta["Q"]
    KC, MC = meta["KC"], meta["MC"]
    groups = meta["groups"]
    SWMAX = meta["SWMAX"]
    pieces_t2 = meta["pieces_t2"]
    psizes = meta["psizes"]
    pbase = meta["pbase"]
    NTAB = meta["NTAB"]
    f32 = mybir.dt.float32
    f32r = mybir.dt.float32r
    bf16 = mybir.dt.bfloat16
    i16 = mybir.dt.int16

    nc = bacc.Bacc(
        "TRN2", target_bir_lowering=False, debug=False,
        num_devices=(1 if single else ncores),
        dynamic_dma_scratch_size=32768,
    )

    h0b_t = nc.dram_tensor("h0b", [N, H], bf16, kind="ExternalInput")
    srcW1_t = nc.dram_tensor("srcW1", [P, NW * 8], i16, kind="ExternalInput")
    srcW2_t = nc.dram_tensor("srcW2", [P, NW * 8], i16, kind="ExternalInput")
    dqT_t = nc.dram_tensor("dqT", [P, NOV], f32, kind="ExternalInput")
    normT_t = nc.dram_tensor("normT", [P, NOV], f32, kind="ExternalInput")
    x0T_t = nc.dram_tensor("x0T", [P, 2 * T2 * T2SZ], f32r, kind="ExternalInput")
    WQ = Q * P // 16
    drugsW_t = nc.dram_tensor("drugsW", [P, WQ], i16, kind="ExternalInput")
    targetsW_t = nc.dram_tensor("targetsW", [P, WQ], i16, kind="ExternalInput")
    iota64_t = nc.dram_tensor("iota64", [P, SWMAX], bf16, kind="ExternalInput")
    wblk_t = nc.dram_tensor("wblk", [P, 2 * R * 2 * P], f32r, kind="ExternalInput")
    loopw_t = nc.dram_tensor("loopw", [P, 2 * 2 * H], f32r, kind="ExternalInput")
    biasT_t = nc.dram_tensor("biasT", [P, 4], f32, kind="ExternalInput")
    fc1_t = nc.dram_tensor("fc1", [P, KC * MC * P], f32r, kind="ExternalInput")
    fc1b_t = nc.dram_tensor("fc1b", [P, MC], f32, kind="ExternalInput")
    fc2_t = nc.dram_tensor("fc2", [P, MC], f32r, kind="ExternalInput")
    out_t = nc.dram_tensor("out", [Q * P, 1], f32, kind="ExternalOutput")

    with tile.TileContext(nc) as tc:
        with (
            tc.tile_pool(name="const", bufs=1) as cp,
            tc.tile_pool(name="work", bufs=2) as wp,
            tc.tile_pool(name="ps", bufs=1, space="PSUM") as pp,
        ):
            # allgathered tables in piece-major layout (no fold pass)
            tab_space = "Local" if single else "Shared"
            h1tab = nc.dram_tensor(
                "h1tab", [NTAB, H], bf16, kind="Internal", addr_space=tab_space
            ).ap()
            h2tab = nc.dram_tensor(
                "h2tab", [NTAB, H], bf16, kind="Internal", addr_space=tab_space
            ).ap()
            agin = {}
            for li in (1, 2):
                for pi, s in enumerate(psizes):
                    agin[(li, pi)] = nc.dram_tensor(
                        f"h{li}_agin{pi}", [s, H], bf16,
                        kind="Internal",
                    ).ap()

            # ---- resident constants, ordered so the DMA bus serves the
            # scatter-critical data first (the bus is FIFO in arrival order;
            # a single big upload ahead of the first gather delays all of
            # layer 1). Weights stream in per (layer, rel-pair) slices just
            # ahead of their first use.
            srcW1 = cp.tile([P, NW * 8], i16, name="srcW1")
            c0 = min(NW * 8, 33 * 8)
            nc.sync.dma_start(srcW1[:, :c0], srcW1_t.ap()[:, :c0])
            iota_sb = cp.tile([P, SWMAX], bf16, name="iota_sb")
            nc.sync.dma_start(iota_sb[:], iota64_t.ap()[:])
            dqT = cp.tile([P, NOV], f32, name="dqT")
            k0 = min(NOV, 96)
            nc.sync.dma_start(dqT[:, :k0], dqT_t.ap()[:, :k0])
            normT = cp.tile([P, NOV], f32, name="normT")
            nc.sync.dma_start(normT[:, :k0], normT_t.ap()[:, :k0])
            nc.sync.dma_start(srcW1[:, c0:], srcW1_t.ap()[:, c0:])
            nc.sync.dma_start(dqT[:, k0:], dqT_t.ap()[:, k0:])
            nc.sync.dma_start(normT[:, k0:], normT_t.ap()[:, k0:])
            wblk_sb = cp.tile([P, 2 * R * 2 * P], f32r, name="wblk_sb")
            for rp in range(R // 2):
                o = ((0 * R + 2 * rp) * 2) * P
                nc.sync.dma_start(
                    wblk_sb[:, o : o + 4 * P], wblk_t.ap()[:, o : o + 4 * P]
                )
            loopw_sb = cp.tile([P, 2 * 2 * H], f32r, name="loopw_sb")
            nc.sync.dma_start(loopw_sb[:, : 2 * H], loopw_t.ap()[:, : 2 * H])
            x0T = cp.tile([P, 2 * T2 * T2SZ], f32r, name="x0T")
            xc0 = 3 * T2SZ
            for hh in range(2):
                o = hh * T2 * T2SZ
                nc.sync.dma_start(
                    x0T[:, o : o + xc0], x0T_t.ap()[:, o : o + xc0]
                )
            biasT_sb = cp.tile([P, 4], f32, name="biasT_sb")
            nc.sync.dma_start(biasT_sb[:], biasT_t.ap()[:])
            ident = cp.tile([P, P], f32, name="ident")
            make_identity(nc, ident[:])
            identr = cp.tile([P, P], f32r, name="identr")
            nc.scalar.copy(identr[:], ident[:])
            identb = cp.tile([P, P], bf16, name="identb")
            nc.scalar.copy(identb[:], ident[:])
            srcW2 = cp.tile([P, NW * 8], i16, name="srcW2")
            nc.sync.dma_start(srcW2[:, :c0], srcW2_t.ap()[:, :c0])

            def _late_consts():
                for hh in range(2):
                    o = hh * T2 * T2SZ
                    nc.sync.dma_start(
                        x0T[:, o + xc0 : o + T2 * T2SZ],
                        x0T_t.ap()[:, o + xc0 : o + T2 * T2SZ],
                    )
                nc.sync.dma_start(srcW2[:, c0:], srcW2_t.ap()[:, c0:])
            drugsW = cp.tile([P, WQ], i16, name="drugsW")
            targetsW = cp.tile([P, WQ], i16, name="targetsW")
            fc1_sb = cp.tile([P, KC * MC * P], f32r, name="fc1_sb")
            fc1b_sb = cp.tile([P, MC], f32, name="fc1b_sb")
            fc2_sb = cp.tile([P, MC], f32r, name="fc2_sb")
            # layer-2 self-loop source, written during layer 1
            h1T = cp.tile([P, 2 * T2 * T2SZ], f32r, name="h1T")

            def wblk_ap(l, r, h):
                o = ((l * R + r) * 2 + h) * P
                return wblk_sb[:, o : o + P]

            def loopw_ap(l, h):
                o = (l * 2 + h) * H
                return loopw_sb[:, o : o + H]

            def emit_ag_piece(li, pi):
                tab = h1tab if li == 1 else h2tab
                r0, r1 = int(pbase[pi]), int(pbase[pi + 1])
                if single:
                    nc.sync.dma_start(
                        tab[r0 : r0 + psizes[pi], :], agin[(li, pi)][:]
                    )
                    return
                nc.gpsimd.collective_compute(
                    "AllGather", mybir.AluOpType.bypass,
                    replica_groups=[list(range(ncores))],
                    ins=[agin[(li, pi)]], outs=[tab[r0:r1, :]],
                )

            copyctr = [0]

            def psum_copy(dst_ap, src_ap, dve_every=3):
                i = copyctr[0]
                copyctr[0] += 1
                if i % dve_every == 0:
                    nc.vector.tensor_copy(dst_ap, src_ap)
                else:
                    nc.scalar.copy(dst_ap, src_ap)

            def layer(l, table_ap, srcW_sb, xT_sb, h1T_out, li, post_t2_hook=None):
                # gather call boundaries: small first calls so the first
                # scatter starts ~one-window-latency after the layer can run
                starts = [0, 1, 4]
                while starts[-1] < NW:
                    starts.append(min(NW, starts[-1] + G))
                while starts[-1] >= NW:
                    starts.pop()
                ncalls = len(starts)
                starts.append(NW)
                w2call = np.searchsorted(starts, np.arange(NW), side="right") - 1
                gtiles = {}

                def issue_gather(ci):
                    if ci in gtiles or ci >= ncalls:
                        return
                    w0 = starts[ci]
                    w = starts[ci + 1] - w0
                    xgw = wp.tile([P, G * H], bf16, name="xgw", tag="xgw", bufs=NXGW)
                    nc.gpsimd.dma_gather(
                        xgw[:, : w * H].rearrange("p (b e) -> p b e", e=H),
                        table_ap,
                        srcW_sb[:, w0 * 8 : (w0 + w) * 8],
                        w * P, w * P, H,
                    )
                    gtiles[ci] = xgw

                def xg(w):
                    ci = int(w2call[w])
                    issue_gather(ci)
                    return gtiles[ci][:, (w - starts[ci]) * H : (w - starts[ci] + 1) * H]

                def prefetch_gathers(gi):
                    # evict calls before this group's first window, then keep
                    # exactly NXGW calls in flight (group + lookahead). Keeps
                    # live tiles <= bufs so pool rotation can never deadlock.
                    wmin = min(ov["w"] for ov in groups[gi])
                    cmin = int(w2call[wmin])
                    for key in [k for k in gtiles if k < cmin]:
                        gtiles.pop(key)
                    for ci in range(cmin, min(cmin + NXGW, ncalls)):
                        issue_gather(ci)

                stiles = {}

                def issue_S(k, span):
                    if k in stiles or k >= NOV:
                        return
                    S = wp.tile([P, SWMAX], bf16, name="S", tag="S", bufs=NSBUF)
                    nc.vector.tensor_scalar(
                        out=S[:, :span], in0=iota_sb[:, :span],
                        scalar1=dqT[:, k : k + 1],
                        scalar2=normT[:, k : k + 1],
                        op0=mybir.AluOpType.is_equal, op1=mybir.AluOpType.mult,
                    )
                    stiles[k] = S

                def issue_group_S(gi):
                    if gi >= len(groups):
                        return
                    for ov in groups[gi]:
                        issue_S(ov["k"], ov["span"])

                pending = None
                first_flush = [True]

                def flush_pending(stop):
                    nonlocal pending
                    if pending is None:
                        return
                    rp_, sbs, msg_ps = pending
                    for s in range(2):
                        r = 2 * rp_ + s
                        for h in range(2):
                            nc.tensor.matmul(
                                msg_ps[h][:],
                                lhsT=wblk_ap(l, r, h),
                                rhs=sbs[h][:, s * T2SZ : (s + 1) * T2SZ],
                                start=first_flush[0] and s == 0,
                                stop=stop and s == 1,
                            )
                    first_flush[0] = False
                    pending = None

                deferred_tail = [None]

                for t2 in range(T2):
                    if t2 == 1 and post_t2_hook is not None:
                        post_t2_hook()
                    msg_ps = {
                        h: pp.tile([P, T2SZ], f32, name=f"m{h}", tag=f"m{h}", bufs=1)
                        for h in range(2)
                    }
                    # layer-1 output stays resident for the next layer's
                    # self-loop; layer-2 output is transient per t2
                    if li == 1:
                        hT_sl = lambda ho, _t2=t2: h1T_out[
                            :, (ho * T2 + _t2) * T2SZ : (ho * T2 + _t2 + 1) * T2SZ
                        ]
                    else:
                        hT_t2 = wp.tile(
                            [P, 2 * T2SZ], f32r, name="h2t", tag="h2t", bufs=2
                        )
                        hT_sl = lambda ho, _t=hT_t2: _t[:, ho * T2SZ : (ho + 1) * T2SZ]
                    first_flush[0] = True
                    for rp in range(RP):
                        gi = t2 * RP + rp
                        prefetch_gathers(gi)
                        issue_group_S(gi)
                        issue_group_S(gi + 1)
                        issue_group_S(gi + 2)
                        agg_ps = {
                            h: pp.tile(
                                [P, 2 * T2SZ], f32, name=f"agg{h}",
                                tag=f"agg{h}", bufs=2,
                            )
                            for h in range(2)
                        }
                        ks = []
                        for h in range(2):
                            for ov in groups[gi]:
                                w, k, fb = ov["w"], ov["k"], ov["fb"]
                                xgt = xg(w)
                                if h == 0:
                                    ks.append(k)
                                S = stiles[k]
                                for (c0s, c1s, st, sp) in ov["parts"]:
                                    nc.tensor.matmul(
                                        agg_ps[h][:, fb * QN + c0s : fb * QN + c1s],
                                        lhsT=xgt[:, h * P : (h + 1) * P],
                                        rhs=S[:, c0s:c1s],
                                        start=st, stop=sp,
                                        skip_group_check=True,
                                    )
                        for k in ks:
                            stiles.pop(k)
                        sbs = {}
                        for h in range(2):
                            aggsb = wp.tile(
                                [P, 2 * T2SZ], f32r, name=f"aggsb{h}",
                                tag=f"aggsb{h}", bufs=3,
                            )
                            psum_copy(aggsb[:], agg_ps[h][:])
                            sbs[h] = aggsb
                        flush_pending(stop=False)
                        pending = (rp, sbs, msg_ps)
                        # previous tile's stores ride behind this tile's
                        # first scatter so its PSUM copies overlap PE work
                        if rp == 0 and deferred_tail[0] is not None:
                            deferred_tail[0]()
                            deferred_tail[0] = None
                    # self-loop emitted before the final flush so it covers
                    # the last group's PSUM->SBUF copy latency
                    for ho in range(2):
                        for h in range(2):
                            nc.tensor.matmul(
                                msg_ps[ho][:],
                                lhsT=loopw_ap(l, h)[:, ho * P : (ho + 1) * P],
                                rhs=xT_sb[:, (h * T2 + t2) * T2SZ : (h * T2 + t2 + 1) * T2SZ],
                                start=False, stop=False,
                            )
                    flush_pending(stop=True)

                    def tail(t2=t2, msg_ps=msg_ps, hT_sl=hT_sl):
                        # msgT -> hT (bias folded into the copy)
                        for ho in range(2):
                            dsl = hT_sl(ho)
                            if ho == 0:
                                nc.scalar.add(
                                    dsl, msg_ps[ho][:],
                                    biasT_sb[:, l * 2 + ho : l * 2 + ho + 1],
                                )
                            else:
                                nc.vector.tensor_scalar(
                                    out=dsl, in0=msg_ps[ho][:],
                                    scalar1=biasT_sb[:, l * 2 + ho : l * 2 + ho + 1],
                                    scalar2=None, op0=mybir.AluOpType.add,
                                )
                        # transpose back to node-major, store to agin piece
                        pi = min(t2 // 3, len(psizes) - 1)
                        t2_0 = pieces_t2[pi][0]
                        for sub in range(2):
                            rows = min(P, NOWN - t2 * T2SZ - sub * P)
                            if rows <= 0:
                                continue
                            tp_ps = pp.tile([P, H], f32r, name="tp", tag="tpx", bufs=1)
                            for ho in range(2):
                                nc.tensor.transpose(
                                    tp_ps[:, ho * P : (ho + 1) * P],
                                    hT_sl(ho)[:, sub * P : (sub + 1) * P],
                                    identr[:],
                                )
                            odt = bf16
                            outb = wp.tile(
                                [P, H], odt, name=f"outb{li}", tag=f"outb{li}", bufs=2
                            )
                            psum_copy(outb[:], tp_ps[:], dve_every=2)
                            o0 = (t2 - t2_0) * T2SZ + sub * P
                            nc.sync.dma_start(
                                agin[(li, pi)][o0 : o0 + rows, :], outb[:rows, :]
                            )
                        if t2 == pieces_t2[pi][1] - 1:
                            emit_ag_piece(li, pi)

                    if t2 == T2 - 1:
                        tail()
                    else:
                        deferred_tail[0] = tail

            layer(0, h0b_t.ap()[:], srcW1, x0T, h1T, 1, post_t2_hook=_late_consts)
            for rp in range(R // 2):
                o = ((1 * R + 2 * rp) * 2) * P
                nc.sync.dma_start(
                    wblk_sb[:, o : o + 4 * P], wblk_t.ap()[:, o : o + 4 * P]
                )
            nc.sync.dma_start(loopw_sb[:, 2 * H :], loopw_t.ap()[:, 2 * H :])
            nc.sync.dma_start(drugsW[:], drugsW_t.ap()[:])
            nc.sync.dma_start(targetsW[:], targetsW_t.ap()[:])
            nc.sync.dma_start(fc1_sb[:], fc1_t.ap()[:])
            nc.sync.dma_start(fc1b_sb[:], fc1b_t.ap()[:])
            nc.sync.dma_start(fc2_sb[:], fc2_t.ap()[:])
            layer(1, h1tab[:], srcW2, h1T, None, 2)

            # ---- MLP head, two stages: stage A (pairs [0, HSPLIT)) gathers
            # from the pieces-1..N-1 table slice, so it runs while the last
            # piece is still computing; stage B covers [NP2-B0, NP2) (the
            # overlap with A is recomputed -- identical values, keeps every
            # matmul free dim >= 256 for full-rate f32r).
            NP2 = Q * P
            HS = meta["HSPLIT"]
            pbase3 = meta["pbase3"]
            blo = NP2 - max(2 * P, NP2 - HS)  # stage-B column start
            QA = HS // P
            xdr = wp.tile([P, Q * H], bf16, name="xdr", tag="xdr", bufs=1)
            xtg = wp.tile([P, Q * H], bf16, name="xtg", tag="xtg", bufs=1)
            nc.gpsimd.dma_gather(
                xdr[:, : QA * H].rearrange("p (b e) -> p b e", e=H),
                h2tab[:pbase3, :], drugsW[:, : QA * 8], HS, HS, H,
            )
            nc.gpsimd.dma_gather(
                xtg[:, : QA * H].rearrange("p (b e) -> p b e", e=H),
                h2tab[:pbase3, :], targetsW[:, : QA * 8], HS, HS, H,
            )

            xcatT = [
                wp.tile([P, NP2], f32r, name=f"xcT{k}", tag=f"xcT{k}", bufs=1)
                for k in range(KC)
            ]

            def head_transposes(q_range):
                for k in range(KC):
                    src_sb = xdr if k < KC // 2 else xtg
                    kk = k % (KC // 2)
                    for qq in q_range:
                        ttag = "tpx" if (k * Q + qq) % 2 == 0 else "agg1"
                        tp2_ps = pp.tile(
                            [P, P], bf16, name="tp2h", tag=ttag,
                            bufs=(1 if ttag == "tpx" else 2),
                        )
                        nc.tensor.transpose(
                            tp2_ps[:],
                            src_sb[:, qq * H + kk * P : qq * H + (kk + 1) * P],
                            identb[:],
                        )
                        if qq % 2 == 0:
                            nc.vector.tensor_copy(
                                xcatT[k][:, qq * P : (qq + 1) * P], tp2_ps[:]
                            )
                        else:
                            nc.scalar.copy(
                                xcatT[k][:, qq * P : (qq + 1) * P], tp2_ps[:]
                            )

            z_ps = pp.tile([1, NP2], f32, name="z", tag="hz", bufs=1)
            yTrs = []

            def head_fc(c0, c1, first_stage):
                for m in range(MC):
                    if first_stage:
                        yT_ps = pp.tile([P, NP2], f32, name="yT", tag="agg0", bufs=2)
                        yTr = wp.tile([P, NP2], f32r, name="yTr", tag="yTr", bufs=4)
                        yTrs.append((yT_ps, yTr))
                    else:
                        yT_ps, yTr = yTrs[m]
                    for k in range(KC):
                        nc.tensor.matmul(
                            yT_ps[:, c0:c1],
                            lhsT=fc1_sb[:, (k * MC + m) * P : (k * MC + m + 1) * P],
                            rhs=xcatT[k][:, c0:c1],
                            start=(k == 0), stop=(k == KC - 1),
                            skip_group_check=True,
                        )
                    nc.scalar.activation(
                        yTr[:, c0:c1], yT_ps[:, c0:c1],
                        mybir.ActivationFunctionType.Relu,
                        bias=fc1b_sb[:, m : m + 1], scale=1.0,
                    )
                    nc.tensor.matmul(
                        z_ps[:, c0:c1], lhsT=fc2_sb[:, m : m + 1],
                        rhs=yTr[:, c0:c1],
                        start=(m == 0), stop=(m == MC - 1),
                        skip_group_check=True,
                    )

            head_transposes(range(QA))
            head_fc(0, HS, True)
            # stage B: rest of the pairs, gathered from the full table
            nc.gpsimd.dma_gather(
                xdr[:, QA * H :].rearrange("p (b e) -> p b e", e=H),
                h2tab[:], drugsW[:, QA * 8 :], NP2 - HS, NP2 - HS, H,
            )
            nc.gpsimd.dma_gather(
                xtg[:, QA * H :].rearrange("p (b e) -> p b e", e=H),
                h2tab[:], targetsW[:, QA * 8 :], NP2 - HS, NP2 - HS, H,
            )
            head_transposes(range(QA, Q))
            head_fc(blo, NP2, False)
            zs = wp.tile([1, NP2], f32, name="zs", tag="zs", bufs=1)
            nc.scalar.activation(
                zs[:], z_ps[:], mybir.ActivationFunctionType.Sigmoid,
                bias=meta["fc2b"], scale=1.0,
            )
            nc.sync.dma_start(out_t.ap()[:, :], zs[:])
    return nc


_NC_CACHE = []


def kernel(**inputs):
    from concourse import bass_utils

    meta, in_maps = _preprocess(inputs)
    skey = (meta["N"], meta["H"], meta["R"], meta["NW"], meta["NOV"], meta["Q"],
            str(meta["groups"]))
    if _NC_CACHE and _NC_CACHE[0][0] == skey:
        nc = _NC_CACHE[0][1]
    else:
        nc = _build(meta)
        nc.compile()
        _NC_CACHE[:] = [(skey, nc)]
    res = bass_utils.run_bass_kernel_spmd(nc, in_maps, core_ids=list(range(NCORES)))
    outs = []
    for c in range(NCORES):
        o = np.asarray(res.results[c]["out"])
        inv = np.empty_like(o)
        inv[meta["head_perm"][c]] = o
        outs.append(inv)
    out = np.concatenate(outs, axis=0)
    return out.astype(np.float32)


# revision 50
# speedup vs baseline: 1.0136x; 1.0136x over previous
"""Trainium2 Bass kernel for the DTI R-GCN (bdd) model, 8 NeuronCores.

v2 strategy (SPMD, one program, per-core data):
  - dst-shard the graph: core c owns nodes [c*2500, (c+1)*2500). Edges are
    bucketed by (dst-tile-of-256, rel-pair, rel, dst-quarter-of-64); bucket
    sizes are padded to the max over the 8 cores so offsets are compile-time
    and shared (pad slots carry norm=0 so they contribute nothing).
  - the padded edge stream is gathered in contiguous 128-edge windows,
    G windows per indirect-DMA call (amortizes the ~1us SWDGE fixed cost),
    from a bf16 copy of the node features.
  - scatter: per 128-edge window one S matrix [128, 64] per bucket-overlap
    (S[e, d] = norm_e * (iota64 == dq_e), rows outside the bucket zeroed via
    norm=0), and one matmul per (overlap, feature-half) accumulates
    aggT[fi, rel-pair 512] in a full PSUM bank -- free dim 64 so scatter PE
    cost is ~4x lower than 256-wide chunks.
  - per (dst-tile, rel-pair): one [128, 512] PSUM->SBUF copy per fi-half,
    then 2 matmuls per rel apply the block-diagonal W (free dim 256, f32r).
    Rel-apply runs one group late to hide the copy latency.
  - self-loop reads resident transposed features (x0T uploaded by host for
    layer 1; h1T written into SBUF during layer 1 with bias folded into the
    PSUM->SBUF activation copy), so no per-tile loads or transposes.
  - layer outputs AllGather piecewise directly INTO the next layer's
    gather-table layout (piece-major rows), so no DRAM fold pass; gather
    indices are host-remapped to that layout.
  - MLP head is data-parallel over pairs (512 per core, free-dim-512 f32r).
"""
import sys

sys.path.insert(0, "/opt/trn_rl_repo")
import numpy as np
import ml_dtypes

P = 128
QN = 64          # dst quarter width (scatter matmul free dim)
T2SZ = 256       # dst nodes per msgT tile / rel-apply free dim
NCORES = 8
G = 8            # windows per gather call
NXGW = 8         # gather tile bufs
NSBUF = 28       # S tile bufs


def _round_f32r(x):
    u = np.ascontiguousarray(x, np.float32).view(np.uint32)
    u = (u + 0x7FF + ((u >> 12) & 1)) & np.uint32(0xFFFFF000)
    return u.view(np.float32)


def _wrap16(flat):
    assert len(flat) % 16 == 0
    w = np.asarray(flat, np.int16).reshape(-1, 16).T.copy()
    return np.tile(w, (8, 1))


def _preprocess(inputs, ncores=NCORES):
    node_ids = np.asarray(inputs["node_ids"])
    src = np.asarray(inputs["src"])
    dst = np.asarray(inputs["dst"])
    etype = np.asarray(inputs["etype"])
    norm = np.asarray(inputs["norm"]).reshape(-1)
    emb = np.asarray(inputs["emb"], dtype=np.float32)
    drugs = np.asarray(inputs["drugs_index"])
    targets = np.asarray(inputs["targets_index"])

    N = node_ids.shape[0]
    H = emb.shape[1]
    R = int(inputs["w1"].shape[0])
    RP = R // 2
    PAIRS = drugs.shape[0]
    assert N % ncores == 0 and PAIRS % ncores == 0
    NOWN = N // ncores
    T2 = -(-NOWN // T2SZ)
    PPC = PAIRS // ncores
    Q = PPC // P
    NBUK = T2 * R * 4

    # ---- node-position balancing ----
    # We are free to choose which owned node occupies which position in the
    # core's [0, NOWN) range (positions define dst tiles/quarters, the
    # output layout, and the gather-table rows -- all remapped consistently
    # below). Greedily assign nodes to (t2, quarter) bins so each bin's
    # per-relation in-degree is as uniform as possible: bucket sizes are
    # padded to the max over cores, so flattening per-core bucket counts
    # directly shrinks the padded edge stream (gather bytes + PE area).
    NBIN = T2 * 4
    owner = dst // NOWN
    pos_g = np.zeros(N, np.int64)  # node -> position within its core
    cap0 = np.full(NBIN, QN, np.int64)
    tail = NOWN - (T2 - 1) * T2SZ  # rows in the last t2
    for q in range(4):
        cap0[(T2 - 1) * 4 + q] = min(QN, max(0, tail - q * QN))
    for c in range(ncores):
        m = owner == c
        ind = np.zeros((NOWN, R), np.float64)
        np.add.at(ind, (dst[m] - c * NOWN, etype[m]), 1.0)
        tot = ind.sum(axis=0)
        cap = cap0.copy()
        target = np.outer(cap0 / float(NOWN), tot)  # [NBIN, R]
        load = np.zeros((NBIN, R), np.float64)
        order = np.argsort(-ind.sum(axis=1), kind="stable")
        nexti = np.zeros(NBIN, np.int64)
        for n in order:
            score = (load - target) @ ind[n]
            score = np.where(cap > 0, score, np.inf)
            b = int(np.argmin(score))
            load[b] += ind[n]
            cap[b] -= 1
            t2b, qb = b // 4, b % 4
            pos_g[c * NOWN + n] = t2b * T2SZ + qb * QN + nexti[b]
            nexti[b] += 1

    # ---- edge bucketing: b = ((t2*RP + rp)*2 + s)*4 + q ----
    d = pos_g[dst]
    t2_e = d // T2SZ
    q_e = (d % T2SZ) // QN
    dq_e = (d % QN).astype(np.float32)
    rp_e = etype // 2
    s_e = etype % 2
    b_e = ((t2_e * RP + rp_e) * 2 + s_e) * 4 + q_e

    counts = np.zeros((ncores, NBUK), np.int64)
    for c in range(ncores):
        counts[c] = np.bincount(b_e[owner == c], minlength=NBUK)
    sz = np.maximum(counts.max(axis=0), 1)
    off = np.zeros(NBUK, np.int64)
    off[1:] = np.cumsum(sz)[:-1]
    TE = int(off[-1] + sz[-1])
    NW = -(-TE // P)
    TEp = NW * P

    # per-core padded slot arrays
    slot_src = np.zeros((ncores, TEp), np.int32)
    slot_dq = np.zeros((ncores, TEp), np.float32)
    slot_norm = np.zeros((ncores, TEp), np.float32)
    for c in range(ncores):
        m = owner == c
        eidx = np.where(m)[0]
        bb = b_e[eidx]
        order = np.argsort(bb, kind="stable")
        eidx = eidx[order]
        bb = bb[order]
        cstart = np.zeros(NBUK, np.int64)
        cstart[1:] = np.cumsum(counts[c])[:-1]
        rank = np.arange(len(eidx)) - cstart[bb]
        pos = off[bb] + rank
        slot_src[c, pos] = src[eidx]
        slot_dq[c, pos] = dq_e[eidx]
        slot_norm[c, pos] = norm[eidx]

    # ---- overlap enumeration at (window x group) granularity ----
    # Buckets of a (t2, rel-pair) group that fall in the same 128-edge
    # window share ONE S matrix: S columns are bucket-relative
    # (dq' = (bucket_pos - first_bucket)*64 + dq), so each edge row's
    # single nonzero lands in its own bucket's 64-column range -- no
    # masking between buckets of the group is needed. Rows outside the
    # group (window crossing a group boundary) are masked via norm=0.
    # Each overlap becomes 1-2 matmuls: a continuation part (the overlap's
    # first bucket continuing from the previous window; start=False) and a
    # fresh part (buckets starting in this window; start=True).
    # groups[t2*RP+rp] = list of overlap dicts
    NGRP = T2 * RP
    groups = [[] for _ in range(NGRP)]
    ov_info = []  # (w, glo, ghi) rows of window belonging to this overlap
    SWMAX = 0
    for g in range(NGRP):
        b0, b1 = g * 8, g * 8 + 8  # bucket range of group
        glo, ghi = int(off[b0]), int(off[b1 - 1] + sz[b1 - 1])
        for w in range(glo // P, (ghi - 1) // P + 1):
            wlo, whi = max(glo, w * P), min(ghi, (w + 1) * P)
            # buckets intersecting [wlo, whi)
            bs = [
                b for b in range(b0, b1)
                if off[b] < whi and off[b] + sz[b] > wlo
            ]
            fb = bs[0] - b0  # first bucket pos in group (0..7)
            span = (bs[-1] - bs[0] + 1) * QN
            SWMAX = max(SWMAX, span)
            k = len(ov_info)
            ov_info.append((w, wlo, whi))
            cont = off[bs[0]] < wlo  # first bucket started earlier
            parts = []
            if cont:
                stop0 = (off[bs[0]] + sz[bs[0]]) <= whi
                parts.append((0, QN, False, stop0))  # S cols, start, stop
                if len(bs) > 1:
                    stop1 = (off[bs[-1]] + sz[bs[-1]]) <= whi
                    parts.append((QN, span, True, stop1))
            else:
                stop1 = (off[bs[-1]] + sz[bs[-1]]) <= whi
                parts.append((0, span, True, stop1))
            groups[g].append(dict(w=w, k=k, fb=fb, span=span, parts=parts))
    NOV = len(ov_info)
    # iota/dq' compare runs in bf16, exact only for integers <= 256
    assert SWMAX <= 256, f"S span {SWMAX} exceeds bf16-exact range"

    # per-core overlap columns: dq' with bucket-relative column offset,
    # norm masked to the group's rows
    dqT = np.zeros((ncores, P, NOV), np.float32)
    normT = np.zeros((ncores, P, NOV), np.float32)
    # per-slot bucket pos within its group (0..7), from offsets
    slot_bpos = np.zeros(TEp, np.int64)
    for b in range(NBUK):
        slot_bpos[off[b] : off[b] + sz[b]] = b % 8
    rows = np.arange(P)
    for g in range(NGRP):
        for ov in groups[g]:
            w, k = ov["w"], ov["k"]
            _, wlo, whi = ov_info[k]
            sl = slice(w * P, (w + 1) * P)
            mask = (rows >= (wlo - w * P)) & (rows < (whi - w * P))
            first_b = ov["fb"]
            rel = (slot_bpos[sl] - first_b) * QN
            for c in range(ncores):
                dqT[c, :, k] = (slot_dq[c, sl] + rel) * mask
                normT[c, :, k] = slot_norm[c, sl] * mask

    # ---- allgather piece structure: pieces of 3 dst-tiles (t2 groups) ----
    # piece p covers t2 [3p, min(3p+3, T2)); rows per t2 = 256 (last: rem)
    pieces_t2 = []
    t20 = 0
    while t20 < T2:
        t21 = min(t20 + 3, T2)
        r0 = t20 * T2SZ
        r1 = min(NOWN, t21 * T2SZ)
        pieces_t2.append((t20, t21, r1 - r0))
        t20 = t21
    NP_ = len(pieces_t2)
    psizes = [pz for (_, _, pz) in pieces_t2]
    pbase = np.zeros(NP_ + 1, np.int64)
    pbase[1:] = np.cumsum([ncores * s for s in psizes])
    NTAB = int(pbase[-1])

    # node id -> piece-layout row in the allgathered table
    def piecemap(n):
        n = np.asarray(n, np.int64)
        c2 = n // NOWN
        rr = pos_g[n]
        t2i = rr // T2SZ
        p = np.minimum(t2i // 3, NP_ - 1)
        szp = np.asarray(psizes, np.int64)[p]
        start = np.asarray([a * 3 * T2SZ for a in range(NP_)], np.int64)[p]
        return (pbase[p] + c2 * szp + (rr - start)).astype(np.int32)

    # gather index uploads
    srcW1 = np.stack([_wrap16(slot_src[c]) for c in range(ncores)])
    src2 = piecemap(slot_src)  # [ncores, TEp]
    srcW2 = np.stack([_wrap16(src2[c]) for c in range(ncores)])
    # head pair reorder: pairs whose drug/target row falls in the last
    # allgather piece go in the final 128 slots, so the first 384 pairs can
    # gather from the pieces-1..3 table slice while the last piece computes
    pbase3 = int(pbase[NP_ - 1])
    head_perm = np.zeros((ncores, PPC), np.int64)
    drows = np.zeros((ncores, PPC), np.int32)
    trows = np.zeros((ncores, PPC), np.int32)
    for c in range(ncores):
        dr = piecemap(drugs[c * PPC : (c + 1) * PPC])
        tr = piecemap(targets[c * PPC : (c + 1) * PPC])
        late = (dr >= pbase3) | (tr >= pbase3)
        perm = np.argsort(late, kind="stable")
        head_perm[c] = perm
        drows[c] = dr[perm]
        trows[c] = tr[perm]
    nlate = np.array(
        [((drows[c] >= pbase3) | (trows[c] >= pbase3)).sum() for c in range(ncores)]
    )
    HSPLIT = (3 if nlate.max() <= P else 2) * P  # stage-A pair count
    drugsW = np.stack([_wrap16(drows[c]) for c in range(ncores)])
    targetsW = np.stack([_wrap16(trows[c]) for c in range(ncores)])

    # ---- features ----
    h0 = emb[node_ids]  # [N, H]
    h0b = h0.astype(ml_dtypes.bfloat16)
    # resident transposed own features: x0T[p, h*(T2*T2SZ) + t2*T2SZ + dd]
    x0T = np.zeros((ncores, P, 2 * T2 * T2SZ), np.float32)
    for c in range(ncores):
        pad = np.zeros((T2 * T2SZ, H), np.float32)
        pad[pos_g[c * NOWN : (c + 1) * NOWN]] = h0[c * NOWN : (c + 1) * NOWN]
        x0T[c] = _round_f32r(
            pad.reshape(T2 * T2SZ, 2, P).transpose(2, 1, 0).reshape(P, 2 * T2 * T2SZ)
        )

    # ---- weights ----
    B = int(inputs["w1"].shape[1])
    si = H // B
    hb = P // si
    wblk = np.zeros((2, R, 2, P, P), np.float32)
    for l, W in enumerate([inputs["w1"], inputs["w2"]]):
        W = np.asarray(W, np.float32)
        for r in range(R):
            for hh in range(2):
                for bb in range(hb):
                    bidx = hb * hh + bb
                    wblk[l, r, hh, bb * si : (bb + 1) * si, bb * si : (bb + 1) * si] = (
                        W[r, bidx]
                    )
    wblk_in = _round_f32r(wblk.transpose(3, 0, 1, 2, 4).reshape(P, 2 * R * 2 * P))
    loopw = np.stack(
        [np.asarray(inputs["loop_w1"], np.float32), np.asarray(inputs["loop_w2"], np.float32)]
    )
    loopw_in = _round_f32r(
        loopw.reshape(2, 2, P, H).transpose(2, 0, 1, 3).reshape(P, 2 * 2 * H)
    )
    biasT = np.zeros((P, 4), np.float32)  # col l*2 + ho
    for l, bkey in enumerate(["b1", "b2"]):
        bv = np.asarray(inputs[bkey], np.float32)
        for ho in range(2):
            biasT[:, l * 2 + ho] = bv[ho * P : (ho + 1) * P]

    d2 = 2 * H
    KC = d2 // P
    MC = d2 // P
    fc1_in = _round_f32r(
        np.asarray(inputs["fc1_W"], np.float32)
        .reshape(KC, P, MC, P)
        .transpose(1, 0, 2, 3)
        .reshape(P, KC * MC * P)
    )
    fc1b_in = np.asarray(inputs["fc1_b"], np.float32).reshape(MC, P).T.copy()
    fc2_in = _round_f32r(np.asarray(inputs["fc2_W"], np.float32).reshape(MC, P).T)
    fc2b = float(np.asarray(inputs["fc2_b"]).reshape(-1)[0])

    iota64 = np.tile(np.arange(SWMAX, dtype=ml_dtypes.bfloat16), (P, 1))

    meta = dict(
        N=N, H=H, R=R, RP=RP, NOWN=NOWN, T2=T2, NW=NW, NOV=NOV, Q=Q,
        KC=KC, MC=MC, fc2b=fc2b, groups=groups, pieces_t2=pieces_t2,
        psizes=psizes, pbase=pbase, NTAB=NTAB, SWMAX=SWMAX,
        HSPLIT=HSPLIT, head_perm=head_perm, pbase3=pbase3,
    )
    shared = dict(
        h0b=h0b, iota64=iota64, wblk=wblk_in, loopw=loopw_in, biasT=biasT,
        fc1=fc1_in, fc1b=fc1b_in, fc2=fc2_in,
    )
    in_maps = []
    for c in range(ncores):
        m = dict(shared)
        m.update(
            srcW1=srcW1[c], srcW2=srcW2[c], dqT=dqT[c], normT=normT[c],
            x0T=x0T[c], drugsW=drugsW[c], targetsW=targetsW[c],
        )
        in_maps.append(m)
    return meta, in_maps


def _build(meta, ncores=NCORES, single=False):
    from concourse import bass, mybir, tile, bacc
    from concourse.masks import make_identity

    N, H, R, RP = meta["N"], meta["H"], meta["R"], meta["RP"]
    NOWN, T2, NW, NOV, Q = meta["NOWN"], meta["T2"], meta["NW"], meta["NOV"], meta["Q"]
    KC, MC = meta["KC"], meta["MC"]
    groups = meta["groups"]
    SWMAX = meta["SWMAX"]
    pieces_t2 = meta["pieces_t2"]
    psizes = meta["psizes"]
    pbase = meta["pbase"]
    NTAB = meta["NTAB"]
    f32 = mybir.dt.float32
    f32r = mybir.dt.float32r
    bf16 = mybir.dt.bfloat16
    i16 = mybir.dt.int16

    nc = bacc.Bacc(
        "TRN2", target_bir_lowering=False, debug=False,
        num_devices=(1 if single else ncores),
        dynamic_dma_scratch_size=32768,
    )

    h0b_t = nc.dram_tensor("h0b", [N, H], bf16, kind="ExternalInput")
    srcW1_t = nc.dram_tensor("srcW1", [P, NW * 8], i16, kind="ExternalInput")
    srcW2_t = nc.dram_tensor("srcW2", [P, NW * 8], i16, kind="ExternalInput")
    dqT_t = nc.dram_tensor("dqT", [P, NOV], f32, kind="ExternalInput")
    normT_t = nc.dram_tensor("normT", [P, NOV], f32, kind="ExternalInput")
    x0T_t = nc.dram_tensor("x0T", [P, 2 * T2 * T2SZ], f32r, kind="ExternalInput")
    WQ = Q * P // 16
    drugsW_t = nc.dram_tensor("drugsW", [P, WQ], i16, kind="ExternalInput")
    targetsW_t = nc.dram_tensor("targetsW", [P, WQ], i16, kind="ExternalInput")
    iota64_t = nc.dram_tensor("iota64", [P, SWMAX], bf16, kind="ExternalInput")
    wblk_t = nc.dram_tensor("wblk", [P, 2 * R * 2 * P], f32r, kind="ExternalInput")
    loopw_t = nc.dram_tensor("loopw", [P, 2 * 2 * H], f32r, kind="ExternalInput")
    biasT_t = nc.dram_tensor("biasT", [P, 4], f32, kind="ExternalInput")
    fc1_t = nc.dram_tensor("fc1", [P, KC * MC * P], f32r, kind="ExternalInput")
    fc1b_t = nc.dram_tensor("fc1b", [P, MC], f32, kind="ExternalInput")
    fc2_t = nc.dram_tensor("fc2", [P, MC], f32r, kind="ExternalInput")
    out_t = nc.dram_tensor("out", [Q * P, 1], f32, kind="ExternalOutput")

    with tile.TileContext(nc) as tc:
        with (
            tc.tile_pool(name="const", bufs=1) as cp,
            tc.tile_pool(name="work", bufs=2) as wp,
            tc.tile_pool(name="ps", bufs=1, space="PSUM") as pp,
        ):
            # allgathered tables in piece-major layout (no fold pass)
            tab_space = "Local" if single else "Shared"
            h1tab = nc.dram_tensor(
                "h1tab", [NTAB, H], bf16, kind="Internal", addr_space=tab_space
            ).ap()
            h2tab = nc.dram_tensor(
                "h2tab", [NTAB, H], bf16, kind="Internal", addr_space=tab_space
            ).ap()
            agin = {}
            for li in (1, 2):
                for pi, s in enumerate(psizes):
                    agin[(li, pi)] = nc.dram_tensor(
                        f"h{li}_agin{pi}", [s, H], bf16,
                        kind="Internal",
                    ).ap()

            # ---- resident constants, ordered so the DMA bus serves the
            # scatter-critical data first (the bus is FIFO in arrival order;
            # a single big upload ahead of the first gather delays all of
            # layer 1). Weights stream in per (layer, rel-pair) slices just
            # ahead of their first use.
            srcW1 = cp.tile([P, NW * 8], i16, name="srcW1")
            c0 = min(NW * 8, 33 * 8)
            nc.sync.dma_start(srcW1[:, :c0], srcW1_t.ap()[:, :c0])
            iota_sb = cp.tile([P, SWMAX], bf16, name="iota_sb")
            nc.sync.dma_start(iota_sb[:], iota64_t.ap()[:])
            dqT = cp.tile([P, NOV], f32, name="dqT")
            k0 = min(NOV, 96)
            nc.sync.dma_start(dqT[:, :k0], dqT_t.ap()[:, :k0])
            normT = cp.tile([P, NOV], f32, name="normT")
            nc.sync.dma_start(normT[:, :k0], normT_t.ap()[:, :k0])
            wblk_sb = cp.tile([P, 2 * R * 2 * P], f32r, name="wblk_sb")
            nc.sync.dma_start(wblk_sb[:, : 4 * P], wblk_t.ap()[:, : 4 * P])
            nc.sync.dma_start(srcW1[:, c0:], srcW1_t.ap()[:, c0:])
            nc.sync.dma_start(dqT[:, k0:], dqT_t.ap()[:, k0:])
            nc.sync.dma_start(normT[:, k0:], normT_t.ap()[:, k0:])
            for rp in range(1, R // 2):
                o = ((0 * R + 2 * rp) * 2) * P
                nc.sync.dma_start(
                    wblk_sb[:, o : o + 4 * P], wblk_t.ap()[:, o : o + 4 * P]
                )
            loopw_sb = cp.tile([P, 2 * 2 * H], f32r, name="loopw_sb")
            nc.sync.dma_start(loopw_sb[:, : 2 * H], loopw_t.ap()[:, : 2 * H])
            x0T = cp.tile([P, 2 * T2 * T2SZ], f32r, name="x0T")
            xc0 = 3 * T2SZ
            for hh in range(2):
                o = hh * T2 * T2SZ
                nc.sync.dma_start(
                    x0T[:, o : o + xc0], x0T_t.ap()[:, o : o + xc0]
                )
            biasT_sb = cp.tile([P, 4], f32, name="biasT_sb")
            nc.sync.dma_start(biasT_sb[:], biasT_t.ap()[:])
            ident = cp.tile([P, P], f32, name="ident")
            make_identity(nc, ident[:])
            identr = cp.tile([P, P], f32r, name="identr")
            nc.scalar.copy(identr[:], ident[:])
            identb = cp.tile([P, P], bf16, name="identb")
            nc.scalar.copy(identb[:], ident[:])
            srcW2 = cp.tile([P, NW * 8], i16, name="srcW2")
            nc.sync.dma_start(srcW2[:, :c0], srcW2_t.ap()[:, :c0])

            def _late_consts():
                for hh in range(2):
                    o = hh * T2 * T2SZ
                    nc.sync.dma_start(
                        x0T[:, o + xc0 : o + T2 * T2SZ],
                        x0T_t.ap()[:, o + xc0 : o + T2 * T2SZ],
                    )
                nc.sync.dma_start(srcW2[:, c0:], srcW2_t.ap()[:, c0:])
            drugsW = cp.tile([P, WQ], i16, name="drugsW")
            targetsW = cp.tile([P, WQ], i16, name="targetsW")
            fc1_sb = cp.tile([P, KC * MC * P], f32r, name="fc1_sb")
            fc1b_sb = cp.tile([P, MC], f32, name="fc1b_sb")
            fc2_sb = cp.tile([P, MC], f32r, name="fc2_sb")
            # layer-2 self-loop source, written during layer 1
            h1T = cp.tile([P, 2 * T2 * T2SZ], f32r, name="h1T")

            def wblk_ap(l, r, h):
                o = ((l * R + r) * 2 + h) * P
                return wblk_sb[:, o : o + P]

            def loopw_ap(l, h):
                o = (l * 2 + h) * H
                return loopw_sb[:, o : o + H]

            def emit_ag_piece(li, pi):
                tab = h1tab if li == 1 else h2tab
                r0, r1 = int(pbase[pi]), int(pbase[pi + 1])
                if single:
                    nc.sync.dma_start(
                        tab[r0 : r0 + psizes[pi], :], agin[(li, pi)][:]
                    )
                    return
                nc.gpsimd.collective_compute(
                    "AllGather", mybir.AluOpType.bypass,
                    replica_groups=[list(range(ncores))],
                    ins=[agin[(li, pi)]], outs=[tab[r0:r1, :]],
                )

            copyctr = [0]

            def psum_copy(dst_ap, src_ap, dve_every=3):
                i = copyctr[0]
                copyctr[0] += 1
                if i % dve_every == 0:
                    nc.vector.tensor_copy(dst_ap, src_ap)
                else:
                    nc.scalar.copy(dst_ap, src_ap)

            def layer(l, table_ap, srcW_sb, xT_sb, h1T_out, li, post_t2_hook=None):
                # gather call boundaries: small first calls so the first
                # scatter starts ~one-window-latency after the layer can run
                starts = [0, 1, 4]
                while starts[-1] < NW:
                    starts.append(min(NW, starts[-1] + G))
                while starts[-1] >= NW:
                    starts.pop()
                ncalls = len(starts)
                starts.append(NW)
                w2call = np.searchsorted(starts, np.arange(NW), side="right") - 1
                gtiles = {}

                def issue_gather(ci):
                    if ci in gtiles or ci >= ncalls:
                        return
                    w0 = starts[ci]
                    w = starts[ci + 1] - w0
                    xgw = wp.tile([P, G * H], bf16, name="xgw", tag="xgw", bufs=NXGW)
                    nc.gpsimd.dma_gather(
                        xgw[:, : w * H].rearrange("p (b e) -> p b e", e=H),
                        table_ap,
                        srcW_sb[:, w0 * 8 : (w0 + w) * 8],
                        w * P, w * P, H,
                    )
                    gtiles[ci] = xgw

                def xg(w):
                    ci = int(w2call[w])
                    issue_gather(ci)
                    return gtiles[ci][:, (w - starts[ci]) * H : (w - starts[ci] + 1) * H]

                def prefetch_gathers(gi):
                    # evict calls before this group's first window, then keep
                    # exactly NXGW calls in flight (group + lookahead). Keeps
                    # live tiles <= bufs so pool rotation can never deadlock.
                    wmin = min(ov["w"] for ov in groups[gi])
                    cmin = int(w2call[wmin])
                    for key in [k for k in gtiles if k < cmin]:
                        gtiles.pop(key)
                    for ci in range(cmin, min(cmin + NXGW, ncalls)):
                        issue_gather(ci)

                stiles = {}

                def issue_S(k, span):
                    if k in stiles or k >= NOV:
                        return
                    S = wp.tile([P, SWMAX], bf16, name="S", tag="S", bufs=NSBUF)
                    nc.vector.tensor_scalar(
                        out=S[:, :span], in0=iota_sb[:, :span],
                        scalar1=dqT[:, k : k + 1],
                        scalar2=normT[:, k : k + 1],
                        op0=mybir.AluOpType.is_equal, op1=mybir.AluOpType.mult,
                    )
                    stiles[k] = S

                def issue_group_S(gi):
                    if gi >= len(groups):
                        return
                    for ov in groups[gi]:
                        issue_S(ov["k"], ov["span"])

                pending = None
                first_flush = [True]

                def flush_pending(stop):
                    nonlocal pending
                    if pending is None:
                        return
                    rp_, sbs, msg_ps = pending
                    for s in range(2):
                        r = 2 * rp_ + s
                        for h in range(2):
                            nc.tensor.matmul(
                                msg_ps[h][:],
                                lhsT=wblk_ap(l, r, h),
                                rhs=sbs[h][:, s * T2SZ : (s + 1) * T2SZ],
                                start=first_flush[0] and s == 0,
                                stop=stop and s == 1,
                            )
                    first_flush[0] = False
                    pending = None

                deferred_tail = [None]

                for t2 in range(T2):
                    if t2 == 1 and post_t2_hook is not None:
                        post_t2_hook()
                    msg_ps = {
                        h: pp.tile([P, T2SZ], f32, name=f"m{h}", tag=f"m{h}", bufs=1)
                        for h in range(2)
                    }
                    # layer-1 output stays resident for the next layer's
                    # self-loop; layer-2 output is transient per t2
                    if li == 1:
                        hT_sl = lambda ho, _t2=t2: h1T_out[
                            :, (ho * T2 + _t2) * T2SZ : (ho * T2 + _t2 + 1) * T2SZ
                        ]
                    else:
                        hT_t2 = wp.tile(
                            [P, 2 * T2SZ], f32r, name="h2t", tag="h2t", bufs=2
                        )
                        hT_sl = lambda ho, _t=hT_t2: _t[:, ho * T2SZ : (ho + 1) * T2SZ]
                    first_flush[0] = True
                    for rp in range(RP):
                        gi = t2 * RP + rp
                        prefetch_gathers(gi)
                        issue_group_S(gi)
                        issue_group_S(gi + 1)
                        issue_group_S(gi + 2)
                        agg_ps = {
                            h: pp.tile(
                                [P, 2 * T2SZ], f32, name=f"agg{h}",
                                tag=f"agg{h}", bufs=2,
                            )
                            for h in range(2)
                        }
                        ks = []
                        for h in range(2):
                            for ov in groups[gi]:
                                w, k, fb = ov["w"], ov["k"], ov["fb"]
                                xgt = xg(w)
                                if h == 0:
                                    ks.append(k)
                                S = stiles[k]
                                for (c0s, c1s, st, sp) in ov["parts"]:
                                    nc.tensor.matmul(
                                        agg_ps[h][:, fb * QN + c0s : fb * QN + c1s],
                                        lhsT=xgt[:, h * P : (h + 1) * P],
                                        rhs=S[:, c0s:c1s],
                                        start=st, stop=sp,
                                        skip_group_check=True,
                                    )
                        for k in ks:
                            stiles.pop(k)
                        sbs = {}
                        for h in range(2):
                            aggsb = wp.tile(
                                [P, 2 * T2SZ], f32r, name=f"aggsb{h}",
                                tag=f"aggsb{h}", bufs=3,
                            )
                            psum_copy(aggsb[:], agg_ps[h][:])
                            sbs[h] = aggsb
                        flush_pending(stop=False)
                        pending = (rp, sbs, msg_ps)
                        # previous tile's stores ride behind this tile's
                        # first scatter so its PSUM copies overlap PE work
                        if rp == 0 and deferred_tail[0] is not None:
                            deferred_tail[0]()
                            deferred_tail[0] = None
                    # self-loop emitted before the final flush so it covers
                    # the last group's PSUM->SBUF copy latency
                    for ho in range(2):
                        for h in range(2):
                            nc.tensor.matmul(
                                msg_ps[ho][:],
                                lhsT=loopw_ap(l, h)[:, ho * P : (ho + 1) * P],
                                rhs=xT_sb[:, (h * T2 + t2) * T2SZ : (h * T2 + t2 + 1) * T2SZ],
                                start=False, stop=False,
                            )
                    flush_pending(stop=True)

                    def tail(t2=t2, msg_ps=msg_ps, hT_sl=hT_sl):
                        # msgT -> hT (bias folded into the copy)
                        for ho in range(2):
                            dsl = hT_sl(ho)
                            if ho == 0:
                                nc.scalar.add(
                                    dsl, msg_ps[ho][:],
                                    biasT_sb[:, l * 2 + ho : l * 2 + ho + 1],
                                )
                            else:
                                nc.vector.tensor_scalar(
                                    out=dsl, in0=msg_ps[ho][:],
                                    scalar1=biasT_sb[:, l * 2 + ho : l * 2 + ho + 1],
                                    scalar2=None, op0=mybir.AluOpType.add,
                                )
                        # transpose back to node-major, store to agin piece
                        pi = min(t2 // 3, len(psizes) - 1)
                        t2_0 = pieces_t2[pi][0]
                        for sub in range(2):
                            rows = min(P, NOWN - t2 * T2SZ - sub * P)
                            if rows <= 0:
                                continue
                            tp_ps = pp.tile([P, H], f32r, name="tp", tag="tpx", bufs=1)
                            for ho in range(2):
                                nc.tensor.transpose(
                                    tp_ps[:, ho * P : (ho + 1) * P],
                                    hT_sl(ho)[:, sub * P : (sub + 1) * P],
                                    identr[:],
                                )
                            odt = bf16
                            outb = wp.tile(
                                [P, H], odt, name=f"outb{li}", tag=f"outb{li}", bufs=2
                            )
                            psum_copy(outb[:], tp_ps[:], dve_every=2)
                            o0 = (t2 - t2_0) * T2SZ + sub * P
                            if single:
                                tab = h1tab if li == 1 else h2tab
                                r0 = int(pbase[pi])
                                nc.sync.dma_start(
                                    tab[r0 + o0 : r0 + o0 + rows, :], outb[:rows, :]
                                )
                            else:
                                nc.sync.dma_start(
                                    agin[(li, pi)][o0 : o0 + rows, :], outb[:rows, :]
                                )
                        if not single and t2 == pieces_t2[pi][1] - 1:
                            emit_ag_piece(li, pi)

                    if t2 == T2 - 1:
                        tail()
                    else:
                        deferred_tail[0] = tail

            layer(0, h0b_t.ap()[:], srcW1, x0T, h1T, 1, post_t2_hook=_late_consts)
            for rp in range(R // 2):
                o = ((1 * R + 2 * rp) * 2) * P
                nc.sync.dma_start(
                    wblk_sb[:, o : o + 4 * P], wblk_t.ap()[:, o : o + 4 * P]
                )
            nc.sync.dma_start(loopw_sb[:, 2 * H :], loopw_t.ap()[:, 2 * H :])
            nc.sync.dma_start(drugsW[:], drugsW_t.ap()[:])
            nc.sync.dma_start(targetsW[:], targetsW_t.ap()[:])
            nc.sync.dma_start(fc1_sb[:], fc1_t.ap()[:])
            nc.sync.dma_start(fc1b_sb[:], fc1b_t.ap()[:])
            nc.sync.dma_start(fc2_sb[:], fc2_t.ap()[:])
            layer(1, h1tab[:], srcW2, h1T, None, 2)

            # ---- MLP head, two stages: stage A (pairs [0, HSPLIT)) gathers
            # from the pieces-1..N-1 table slice, so it runs while the last
            # piece is still computing; stage B covers [NP2-B0, NP2) (the
            # overlap with A is recomputed -- identical values, keeps every
            # matmul free dim >= 256 for full-rate f32r).
            NP2 = Q * P
            HS = meta["HSPLIT"]
            pbase3 = meta["pbase3"]
            blo = NP2 - max(2 * P, NP2 - HS)  # stage-B column start
            QA = HS // P
            xdr = wp.tile([P, Q * H], bf16, name="xdr", tag="xdr", bufs=1)
            xtg = wp.tile([P, Q * H], bf16, name="xtg", tag="xtg", bufs=1)
            nc.gpsimd.dma_gather(
                xdr[:, : QA * H].rearrange("p (b e) -> p b e", e=H),
                h2tab[:pbase3, :], drugsW[:, : QA * 8], HS, HS, H,
            )
            nc.gpsimd.dma_gather(
                xtg[:, : QA * H].rearrange("p (b e) -> p b e", e=H),
                h2tab[:pbase3, :], targetsW[:, : QA * 8], HS, HS, H,
            )

            xcatT = [
                wp.tile([P, NP2], f32r, name=f"xcT{k}", tag=f"xcT{k}", bufs=1)
                for k in range(KC)
            ]

            def head_transposes(q_range):
                for k in range(KC):
                    src_sb = xdr if k < KC // 2 else xtg
                    kk = k % (KC // 2)
                    for qq in q_range:
                        ttag = "tpx" if (k * Q + qq) % 2 == 0 else "agg1"
                        tp2_ps = pp.tile(
                            [P, P], bf16, name="tp2h", tag=ttag,
                            bufs=(1 if ttag == "tpx" else 2),
                        )
                        nc.tensor.transpose(
                            tp2_ps[:],
                            src_sb[:, qq * H + kk * P : qq * H + (kk + 1) * P],
                            identb[:],
                        )
                        if qq % 2 == 0:
                            nc.vector.tensor_copy(
                                xcatT[k][:, qq * P : (qq + 1) * P], tp2_ps[:]
                            )
                        else:
                            nc.scalar.copy(
                                xcatT[k][:, qq * P : (qq + 1) * P], tp2_ps[:]
                            )

            z_ps = pp.tile([1, NP2], f32, name="z", tag="hz", bufs=1)
            yTrs = []

            def head_fc(c0, c1, first_stage):
                for m in range(MC):
                    if first_stage:
                        yT_ps = pp.tile([P, NP2], f32, name="yT", tag="agg0", bufs=2)
                        yTr = wp.tile([P, NP2], f32r, name="yTr", tag="yTr", bufs=4)
                        yTrs.append((yT_ps, yTr))
                    else:
                        yT_ps, yTr = yTrs[m]
                    for k in range(KC):
                        nc.tensor.matmul(
                            yT_ps[:, c0:c1],
                            lhsT=fc1_sb[:, (k * MC + m) * P : (k * MC + m + 1) * P],
                            rhs=xcatT[k][:, c0:c1],
                            start=(k == 0), stop=(k == KC - 1),
                            skip_group_check=True,
                        )
                    nc.scalar.activation(
                        yTr[:, c0:c1], yT_ps[:, c0:c1],
                        mybir.ActivationFunctionType.Relu,
                        bias=fc1b_sb[:, m : m + 1], scale=1.0,
                    )
                    nc.tensor.matmul(
                        z_ps[:, c0:c1], lhsT=fc2_sb[:, m : m + 1],
                        rhs=yTr[:, c0:c1],
                        start=(m == 0), stop=(m == MC - 1),
                        skip_group_check=True,
                    )

            head_transposes(range(QA))
            head_fc(0, HS, True)
            # stage B: rest of the pairs, gathered from the full table
            nc.gpsimd.dma_gather(
                xdr[:, QA * H :].rearrange("p (b e) -> p b e", e=H),
                h2tab[:], drugsW[:, QA * 8 :], NP2 - HS, NP2 - HS, H,
            )
            nc.gpsimd.dma_gather(
                xtg[:, QA * H :].rearrange("p (b e) -> p b e", e=H),
                h2tab[:], targetsW[:, QA * 8 :], NP2 - HS, NP2 - HS, H,
            )
            head_transposes(range(QA, Q))
            head_fc(blo, NP2, False)
            zs = wp.tile([1, NP2], f32, name="zs", tag="zs", bufs=1)
            nc.scalar.activation(
                zs[:], z_ps[:], mybir.ActivationFunctionType.Sigmoid,
                bias=meta["fc2b"], scale=1.0,
            )
            nc.sync.dma_start(out_t.ap()[:, :], zs[:])
    return nc


_NC_CACHE = []


def kernel(**inputs):
    from concourse import bass_utils

    meta, in_maps = _preprocess(inputs)
    skey = (meta["N"], meta["H"], meta["R"], meta["NW"], meta["NOV"], meta["Q"],
            str(meta["groups"]))
    if _NC_CACHE and _NC_CACHE[0][0] == skey:
        nc = _NC_CACHE[0][1]
    else:
        nc = _build(meta)
        nc.compile()
        _NC_CACHE[:] = [(skey, nc)]
    res = bass_utils.run_bass_kernel_spmd(nc, in_maps, core_ids=list(range(NCORES)))
    outs = []
    for c in range(NCORES):
        o = np.asarray(res.results[c]["out"])
        inv = np.empty_like(o)
        inv[meta["head_perm"][c]] = o
        outs.append(inv)
    out = np.concatenate(outs, axis=0)
    return out.astype(np.float32)


# revision 51
# speedup vs baseline: 1.0141x; 1.0005x over previous
"""Trainium2 Bass kernel for the DTI R-GCN (bdd) model, 8 NeuronCores.

v2 strategy (SPMD, one program, per-core data):
  - dst-shard the graph: core c owns nodes [c*2500, (c+1)*2500). Edges are
    bucketed by (dst-tile-of-256, rel-pair, rel, dst-quarter-of-64); bucket
    sizes are padded to the max over the 8 cores so offsets are compile-time
    and shared (pad slots carry norm=0 so they contribute nothing).
  - the padded edge stream is gathered in contiguous 128-edge windows,
    G windows per indirect-DMA call (amortizes the ~1us SWDGE fixed cost),
    from a bf16 copy of the node features.
  - scatter: per 128-edge window one S matrix [128, 64] per bucket-overlap
    (S[e, d] = norm_e * (iota64 == dq_e), rows outside the bucket zeroed via
    norm=0), and one matmul per (overlap, feature-half) accumulates
    aggT[fi, rel-pair 512] in a full PSUM bank -- free dim 64 so scatter PE
    cost is ~4x lower than 256-wide chunks.
  - per (dst-tile, rel-pair): one [128, 512] PSUM->SBUF copy per fi-half,
    then 2 matmuls per rel apply the block-diagonal W (free dim 256, f32r).
    Rel-apply runs one group late to hide the copy latency.
  - self-loop reads resident transposed features (x0T uploaded by host for
    layer 1; h1T written into SBUF during layer 1 with bias folded into the
    PSUM->SBUF activation copy), so no per-tile loads or transposes.
  - layer outputs AllGather piecewise directly INTO the next layer's
    gather-table layout (piece-major rows), so no DRAM fold pass; gather
    indices are host-remapped to that layout.
  - MLP head is data-parallel over pairs (512 per core, free-dim-512 f32r).
"""
import sys

sys.path.insert(0, "/opt/trn_rl_repo")
import numpy as np
import ml_dtypes

P = 128
QN = 64          # dst quarter width (scatter matmul free dim)
T2SZ = 256       # dst nodes per msgT tile / rel-apply free dim
NCORES = 8
G = 8            # windows per gather call
NXGW = 8         # gather tile bufs
NSBUF = 28       # S tile bufs


def _round_f32r(x):
    u = np.ascontiguousarray(x, np.float32).view(np.uint32)
    u = (u + 0x7FF + ((u >> 12) & 1)) & np.uint32(0xFFFFF000)
    return u.view(np.float32)


def _wrap16(flat):
    assert len(flat) % 16 == 0
    w = np.asarray(flat, np.int16).reshape(-1, 16).T.copy()
    return np.tile(w, (8, 1))


def _preprocess(inputs, ncores=NCORES):
    node_ids = np.asarray(inputs["node_ids"])
    src = np.asarray(inputs["src"])
    dst = np.asarray(inputs["dst"])
    etype = np.asarray(inputs["etype"])
    norm = np.asarray(inputs["norm"]).reshape(-1)
    emb = np.asarray(inputs["emb"], dtype=np.float32)
    drugs = np.asarray(inputs["drugs_index"])
    targets = np.asarray(inputs["targets_index"])

    N = node_ids.shape[0]
    H = emb.shape[1]
    R = int(inputs["w1"].shape[0])
    RP = R // 2
    PAIRS = drugs.shape[0]
    assert N % ncores == 0 and PAIRS % ncores == 0
    NOWN = N // ncores
    T2 = -(-NOWN // T2SZ)
    PPC = PAIRS // ncores
    Q = PPC // P
    NBUK = T2 * R * 4

    # ---- node-position balancing ----
    # We are free to choose which owned node occupies which position in the
    # core's [0, NOWN) range (positions define dst tiles/quarters, the
    # output layout, and the gather-table rows -- all remapped consistently
    # below). Greedily assign nodes to (t2, quarter) bins so each bin's
    # per-relation in-degree is as uniform as possible: bucket sizes are
    # padded to the max over cores, so flattening per-core bucket counts
    # directly shrinks the padded edge stream (gather bytes + PE area).
    NBIN = T2 * 4
    owner = dst // NOWN
    pos_g = np.zeros(N, np.int64)  # node -> position within its core
    cap0 = np.full(NBIN, QN, np.int64)
    tail = NOWN - (T2 - 1) * T2SZ  # rows in the last t2
    for q in range(4):
        cap0[(T2 - 1) * 4 + q] = min(QN, max(0, tail - q * QN))
    for c in range(ncores):
        m = owner == c
        ind = np.zeros((NOWN, R), np.float64)
        np.add.at(ind, (dst[m] - c * NOWN, etype[m]), 1.0)
        tot = ind.sum(axis=0)
        cap = cap0.copy()
        target = np.outer(cap0 / float(NOWN), tot)  # [NBIN, R]
        load = np.zeros((NBIN, R), np.float64)
        order = np.argsort(-ind.sum(axis=1), kind="stable")
        nexti = np.zeros(NBIN, np.int64)
        for n in order:
            score = (load - target) @ ind[n]
            score = np.where(cap > 0, score, np.inf)
            b = int(np.argmin(score))
            load[b] += ind[n]
            cap[b] -= 1
            t2b, qb = b // 4, b % 4
            pos_g[c * NOWN + n] = t2b * T2SZ + qb * QN + nexti[b]
            nexti[b] += 1

    # ---- edge bucketing: b = ((t2*RP + rp)*2 + s)*4 + q ----
    d = pos_g[dst]
    t2_e = d // T2SZ
    q_e = (d % T2SZ) // QN
    dq_e = (d % QN).astype(np.float32)
    rp_e = etype // 2
    s_e = etype % 2
    b_e = ((t2_e * RP + rp_e) * 2 + s_e) * 4 + q_e

    counts = np.zeros((ncores, NBUK), np.int64)
    for c in range(ncores):
        counts[c] = np.bincount(b_e[owner == c], minlength=NBUK)
    sz = np.maximum(counts.max(axis=0), 1)
    off = np.zeros(NBUK, np.int64)
    off[1:] = np.cumsum(sz)[:-1]
    TE = int(off[-1] + sz[-1])
    NW = -(-TE // P)
    TEp = NW * P

    # per-core padded slot arrays
    slot_src = np.zeros((ncores, TEp), np.int32)
    slot_dq = np.zeros((ncores, TEp), np.float32)
    slot_norm = np.zeros((ncores, TEp), np.float32)
    for c in range(ncores):
        m = owner == c
        eidx = np.where(m)[0]
        bb = b_e[eidx]
        order = np.argsort(bb, kind="stable")
        eidx = eidx[order]
        bb = bb[order]
        cstart = np.zeros(NBUK, np.int64)
        cstart[1:] = np.cumsum(counts[c])[:-1]
        rank = np.arange(len(eidx)) - cstart[bb]
        pos = off[bb] + rank
        slot_src[c, pos] = src[eidx]
        slot_dq[c, pos] = dq_e[eidx]
        slot_norm[c, pos] = norm[eidx]

    # ---- overlap enumeration at (window x group) granularity ----
    # Buckets of a (t2, rel-pair) group that fall in the same 128-edge
    # window share ONE S matrix: S columns are bucket-relative
    # (dq' = (bucket_pos - first_bucket)*64 + dq), so each edge row's
    # single nonzero lands in its own bucket's 64-column range -- no
    # masking between buckets of the group is needed. Rows outside the
    # group (window crossing a group boundary) are masked via norm=0.
    # Each overlap becomes 1-2 matmuls: a continuation part (the overlap's
    # first bucket continuing from the previous window; start=False) and a
    # fresh part (buckets starting in this window; start=True).
    # groups[t2*RP+rp] = list of overlap dicts
    NGRP = T2 * RP
    groups = [[] for _ in range(NGRP)]
    ov_info = []  # (w, glo, ghi) rows of window belonging to this overlap
    SWMAX = 0
    for g in range(NGRP):
        b0, b1 = g * 8, g * 8 + 8  # bucket range of group
        glo, ghi = int(off[b0]), int(off[b1 - 1] + sz[b1 - 1])
        for w in range(glo // P, (ghi - 1) // P + 1):
            wlo, whi = max(glo, w * P), min(ghi, (w + 1) * P)
            # buckets intersecting [wlo, whi)
            bs = [
                b for b in range(b0, b1)
                if off[b] < whi and off[b] + sz[b] > wlo
            ]
            fb = bs[0] - b0  # first bucket pos in group (0..7)
            span = (bs[-1] - bs[0] + 1) * QN
            SWMAX = max(SWMAX, span)
            k = len(ov_info)
            ov_info.append((w, wlo, whi))
            cont = off[bs[0]] < wlo  # first bucket started earlier
            parts = []
            if cont:
                stop0 = (off[bs[0]] + sz[bs[0]]) <= whi
                parts.append((0, QN, False, stop0))  # S cols, start, stop
                if len(bs) > 1:
                    stop1 = (off[bs[-1]] + sz[bs[-1]]) <= whi
                    parts.append((QN, span, True, stop1))
            else:
                stop1 = (off[bs[-1]] + sz[bs[-1]]) <= whi
                parts.append((0, span, True, stop1))
            groups[g].append(dict(w=w, k=k, fb=fb, span=span, parts=parts))
    NOV = len(ov_info)
    # iota/dq' compare runs in bf16, exact only for integers <= 256
    assert SWMAX <= 256, f"S span {SWMAX} exceeds bf16-exact range"

    # per-core overlap columns: dq' with bucket-relative column offset,
    # norm masked to the group's rows
    dqT = np.zeros((ncores, P, NOV), np.float32)
    normT = np.zeros((ncores, P, NOV), np.float32)
    # per-slot bucket pos within its group (0..7), from offsets
    slot_bpos = np.zeros(TEp, np.int64)
    for b in range(NBUK):
        slot_bpos[off[b] : off[b] + sz[b]] = b % 8
    rows = np.arange(P)
    for g in range(NGRP):
        for ov in groups[g]:
            w, k = ov["w"], ov["k"]
            _, wlo, whi = ov_info[k]
            sl = slice(w * P, (w + 1) * P)
            mask = (rows >= (wlo - w * P)) & (rows < (whi - w * P))
            first_b = ov["fb"]
            rel = (slot_bpos[sl] - first_b) * QN
            for c in range(ncores):
                dqT[c, :, k] = (slot_dq[c, sl] + rel) * mask
                normT[c, :, k] = slot_norm[c, sl] * mask

    # ---- allgather piece structure: pieces of 3 dst-tiles (t2 groups) ----
    # piece p covers t2 [3p, min(3p+3, T2)); rows per t2 = 256 (last: rem)
    pieces_t2 = []
    t20 = 0
    while t20 < T2:
        t21 = min(t20 + 3, T2)
        r0 = t20 * T2SZ
        r1 = min(NOWN, t21 * T2SZ)
        pieces_t2.append((t20, t21, r1 - r0))
        t20 = t21
    NP_ = len(pieces_t2)
    psizes = [pz for (_, _, pz) in pieces_t2]
    pbase = np.zeros(NP_ + 1, np.int64)
    pbase[1:] = np.cumsum([ncores * s for s in psizes])
    NTAB = int(pbase[-1])

    # node id -> piece-layout row in the allgathered table
    def piecemap(n):
        n = np.asarray(n, np.int64)
        c2 = n // NOWN
        rr = pos_g[n]
        t2i = rr // T2SZ
        p = np.minimum(t2i // 3, NP_ - 1)
        szp = np.asarray(psizes, np.int64)[p]
        start = np.asarray([a * 3 * T2SZ for a in range(NP_)], np.int64)[p]
        return (pbase[p] + c2 * szp + (rr - start)).astype(np.int32)

    # gather index uploads
    srcW1 = np.stack([_wrap16(slot_src[c]) for c in range(ncores)])
    src2 = piecemap(slot_src)  # [ncores, TEp]
    srcW2 = np.stack([_wrap16(src2[c]) for c in range(ncores)])
    # head pair reorder: pairs whose drug/target row falls in the last
    # allgather piece go in the final 128 slots, so the first 384 pairs can
    # gather from the pieces-1..3 table slice while the last piece computes
    pbase3 = int(pbase[NP_ - 1])
    head_perm = np.zeros((ncores, PPC), np.int64)
    drows = np.zeros((ncores, PPC), np.int32)
    trows = np.zeros((ncores, PPC), np.int32)
    for c in range(ncores):
        dr = piecemap(drugs[c * PPC : (c + 1) * PPC])
        tr = piecemap(targets[c * PPC : (c + 1) * PPC])
        late = (dr >= pbase3) | (tr >= pbase3)
        perm = np.argsort(late, kind="stable")
        head_perm[c] = perm
        drows[c] = dr[perm]
        trows[c] = tr[perm]
    nlate = np.array(
        [((drows[c] >= pbase3) | (trows[c] >= pbase3)).sum() for c in range(ncores)]
    )
    HSPLIT = (3 if nlate.max() <= P else 2) * P  # stage-A pair count
    drugsW = np.stack([_wrap16(drows[c]) for c in range(ncores)])
    targetsW = np.stack([_wrap16(trows[c]) for c in range(ncores)])

    # ---- features ----
    h0 = emb[node_ids]  # [N, H]
    h0b = h0.astype(ml_dtypes.bfloat16)
    # resident transposed own features: x0T[p, h*(T2*T2SZ) + t2*T2SZ + dd]
    x0T = np.zeros((ncores, P, 2 * T2 * T2SZ), np.float32)
    for c in range(ncores):
        pad = np.zeros((T2 * T2SZ, H), np.float32)
        pad[pos_g[c * NOWN : (c + 1) * NOWN]] = h0[c * NOWN : (c + 1) * NOWN]
        x0T[c] = _round_f32r(
            pad.reshape(T2 * T2SZ, 2, P).transpose(2, 1, 0).reshape(P, 2 * T2 * T2SZ)
        )

    # ---- weights ----
    B = int(inputs["w1"].shape[1])
    si = H // B
    hb = P // si
    wblk = np.zeros((2, R, 2, P, P), np.float32)
    for l, W in enumerate([inputs["w1"], inputs["w2"]]):
        W = np.asarray(W, np.float32)
        for r in range(R):
            for hh in range(2):
                for bb in range(hb):
                    bidx = hb * hh + bb
                    wblk[l, r, hh, bb * si : (bb + 1) * si, bb * si : (bb + 1) * si] = (
                        W[r, bidx]
                    )
    wblk_in = _round_f32r(wblk.transpose(3, 0, 1, 2, 4).reshape(P, 2 * R * 2 * P))
    loopw = np.stack(
        [np.asarray(inputs["loop_w1"], np.float32), np.asarray(inputs["loop_w2"], np.float32)]
    )
    loopw_in = _round_f32r(
        loopw.reshape(2, 2, P, H).transpose(2, 0, 1, 3).reshape(P, 2 * 2 * H)
    )
    biasT = np.zeros((P, 4), np.float32)  # col l*2 + ho
    for l, bkey in enumerate(["b1", "b2"]):
        bv = np.asarray(inputs[bkey], np.float32)
        for ho in range(2):
            biasT[:, l * 2 + ho] = bv[ho * P : (ho + 1) * P]

    d2 = 2 * H
    KC = d2 // P
    MC = d2 // P
    fc1_in = _round_f32r(
        np.asarray(inputs["fc1_W"], np.float32)
        .reshape(KC, P, MC, P)
        .transpose(1, 0, 2, 3)
        .reshape(P, KC * MC * P)
    )
    fc1b_in = np.asarray(inputs["fc1_b"], np.float32).reshape(MC, P).T.copy()
    fc2_in = _round_f32r(np.asarray(inputs["fc2_W"], np.float32).reshape(MC, P).T)
    fc2b = float(np.asarray(inputs["fc2_b"]).reshape(-1)[0])

    iota64 = np.tile(np.arange(SWMAX, dtype=ml_dtypes.bfloat16), (P, 1))

    meta = dict(
        N=N, H=H, R=R, RP=RP, NOWN=NOWN, T2=T2, NW=NW, NOV=NOV, Q=Q,
        KC=KC, MC=MC, fc2b=fc2b, groups=groups, pieces_t2=pieces_t2,
        psizes=psizes, pbase=pbase, NTAB=NTAB, SWMAX=SWMAX,
        HSPLIT=HSPLIT, head_perm=head_perm, pbase3=pbase3,
    )
    shared = dict(
        h0b=h0b, iota64=iota64, wblk=wblk_in, loopw=loopw_in, biasT=biasT,
        fc1=fc1_in, fc1b=fc1b_in, fc2=fc2_in,
    )
    in_maps = []
    for c in range(ncores):
        m = dict(shared)
        m.update(
            srcW1=srcW1[c], srcW2=srcW2[c], dqT=dqT[c], normT=normT[c],
            x0T=x0T[c], drugsW=drugsW[c], targetsW=targetsW[c],
        )
        in_maps.append(m)
    return meta, in_maps


def _build(meta, ncores=NCORES, single=False):
    from concourse import bass, mybir, tile, bacc
    from concourse.masks import make_identity

    N, H, R, RP = meta["N"], meta["H"], meta["R"], meta["RP"]
    NOWN, T2, NW, NOV, Q = meta["NOWN"], meta["T2"], meta["NW"], meta["NOV"], meta["Q"]
    KC, MC = meta["KC"], meta["MC"]
    groups = meta["groups"]
    SWMAX = meta["SWMAX"]
    pieces_t2 = meta["pieces_t2"]
    psizes = meta["psizes"]
    pbase = meta["pbase"]
    NTAB = meta["NTAB"]
    f32 = mybir.dt.float32
    f32r = mybir.dt.float32r
    bf16 = mybir.dt.bfloat16
    i16 = mybir.dt.int16

    nc = bacc.Bacc(
        "TRN2", target_bir_lowering=False, debug=False,
        num_devices=(1 if single else ncores),
        dynamic_dma_scratch_size=32768,
    )

    h0b_t = nc.dram_tensor("h0b", [N, H], bf16, kind="ExternalInput")
    srcW1_t = nc.dram_tensor("srcW1", [P, NW * 8], i16, kind="ExternalInput")
    srcW2_t = nc.dram_tensor("srcW2", [P, NW * 8], i16, kind="ExternalInput")
    dqT_t = nc.dram_tensor("dqT", [P, NOV], f32, kind="ExternalInput")
    normT_t = nc.dram_tensor("normT", [P, NOV], f32, kind="ExternalInput")
    x0T_t = nc.dram_tensor("x0T", [P, 2 * T2 * T2SZ], f32r, kind="ExternalInput")
    WQ = Q * P // 16
    drugsW_t = nc.dram_tensor("drugsW", [P, WQ], i16, kind="ExternalInput")
    targetsW_t = nc.dram_tensor("targetsW", [P, WQ], i16, kind="ExternalInput")
    iota64_t = nc.dram_tensor("iota64", [P, SWMAX], bf16, kind="ExternalInput")
    wblk_t = nc.dram_tensor("wblk", [P, 2 * R * 2 * P], f32r, kind="ExternalInput")
    loopw_t = nc.dram_tensor("loopw", [P, 2 * 2 * H], f32r, kind="ExternalInput")
    biasT_t = nc.dram_tensor("biasT", [P, 4], f32, kind="ExternalInput")
    fc1_t = nc.dram_tensor("fc1", [P, KC * MC * P], f32r, kind="ExternalInput")
    fc1b_t = nc.dram_tensor("fc1b", [P, MC], f32, kind="ExternalInput")
    fc2_t = nc.dram_tensor("fc2", [P, MC], f32r, kind="ExternalInput")
    out_t = nc.dram_tensor("out", [Q * P, 1], f32, kind="ExternalOutput")

    with tile.TileContext(nc) as tc:
        with (
            tc.tile_pool(name="const", bufs=1) as cp,
            tc.tile_pool(name="work", bufs=2) as wp,
            tc.tile_pool(name="ps", bufs=1, space="PSUM") as pp,
        ):
            # allgathered tables in piece-major layout (no fold pass)
            tab_space = "Local" if single else "Shared"
            h1tab = nc.dram_tensor(
                "h1tab", [NTAB, H], bf16, kind="Internal", addr_space=tab_space
            ).ap()
            h2tab = nc.dram_tensor(
                "h2tab", [NTAB, H], bf16, kind="Internal", addr_space=tab_space
            ).ap()
            agin = {}
            for li in (1, 2):
                for pi, s in enumerate(psizes):
                    agin[(li, pi)] = nc.dram_tensor(
                        f"h{li}_agin{pi}", [s, H], bf16,
                        kind="Internal",
                    ).ap()

            # ---- resident constants, ordered so the DMA bus serves the
            # scatter-critical data first (the bus is FIFO in arrival order;
            # a single big upload ahead of the first gather delays all of
            # layer 1). Weights stream in per (layer, rel-pair) slices just
            # ahead of their first use.
            srcW1 = cp.tile([P, NW * 8], i16, name="srcW1")
            c0 = min(NW * 8, 33 * 8)
            nc.sync.dma_start(srcW1[:, :c0], srcW1_t.ap()[:, :c0])
            iota_sb = cp.tile([P, SWMAX], bf16, name="iota_sb")
            nc.sync.dma_start(iota_sb[:], iota64_t.ap()[:])
            dqT = cp.tile([P, NOV], f32, name="dqT")
            k0 = min(NOV, 96)
            nc.sync.dma_start(dqT[:, :k0], dqT_t.ap()[:, :k0])
            normT = cp.tile([P, NOV], f32, name="normT")
            nc.sync.dma_start(normT[:, :k0], normT_t.ap()[:, :k0])
            wblk_sb = cp.tile([P, 2 * R * 2 * P], f32r, name="wblk_sb")
            nc.sync.dma_start(wblk_sb[:, : 4 * P], wblk_t.ap()[:, : 4 * P])
            nc.sync.dma_start(srcW1[:, c0:], srcW1_t.ap()[:, c0:])
            nc.sync.dma_start(dqT[:, k0:], dqT_t.ap()[:, k0:])
            nc.sync.dma_start(normT[:, k0:], normT_t.ap()[:, k0:])
            for rp in range(1, R // 2):
                o = ((0 * R + 2 * rp) * 2) * P
                nc.sync.dma_start(
                    wblk_sb[:, o : o + 4 * P], wblk_t.ap()[:, o : o + 4 * P]
                )
            loopw_sb = cp.tile([P, 2 * 2 * H], f32r, name="loopw_sb")
            nc.sync.dma_start(loopw_sb[:, : 2 * H], loopw_t.ap()[:, : 2 * H])
            x0T = cp.tile([P, 2 * T2 * T2SZ], f32r, name="x0T")
            xc0 = 3 * T2SZ
            for hh in range(2):
                o = hh * T2 * T2SZ
                nc.sync.dma_start(
                    x0T[:, o : o + xc0], x0T_t.ap()[:, o : o + xc0]
                )
            biasT_sb = cp.tile([P, 4], f32, name="biasT_sb")
            nc.sync.dma_start(biasT_sb[:], biasT_t.ap()[:])
            ident = cp.tile([P, P], f32, name="ident")
            make_identity(nc, ident[:])
            identr = cp.tile([P, P], f32r, name="identr")
            nc.scalar.copy(identr[:], ident[:])
            identb = cp.tile([P, P], bf16, name="identb")
            nc.scalar.copy(identb[:], ident[:])
            srcW2 = cp.tile([P, NW * 8], i16, name="srcW2")
            nc.sync.dma_start(srcW2[:, :c0], srcW2_t.ap()[:, :c0])

            def _late_consts():
                for hh in range(2):
                    o = hh * T2 * T2SZ
                    nc.sync.dma_start(
                        x0T[:, o + xc0 : o + T2 * T2SZ],
                        x0T_t.ap()[:, o + xc0 : o + T2 * T2SZ],
                    )
                nc.sync.dma_start(srcW2[:, c0:], srcW2_t.ap()[:, c0:])
            drugsW = cp.tile([P, WQ], i16, name="drugsW")
            targetsW = cp.tile([P, WQ], i16, name="targetsW")
            fc1_sb = cp.tile([P, KC * MC * P], f32r, name="fc1_sb")
            fc1b_sb = cp.tile([P, MC], f32, name="fc1b_sb")
            fc2_sb = cp.tile([P, MC], f32r, name="fc2_sb")
            # layer-2 self-loop source, written during layer 1
            h1T = cp.tile([P, 2 * T2 * T2SZ], f32r, name="h1T")

            def wblk_ap(l, r, h):
                o = ((l * R + r) * 2 + h) * P
                return wblk_sb[:, o : o + P]

            def loopw_ap(l, h):
                o = (l * 2 + h) * H
                return loopw_sb[:, o : o + H]

            def emit_ag_piece(li, pi):
                tab = h1tab if li == 1 else h2tab
                r0, r1 = int(pbase[pi]), int(pbase[pi + 1])
                if single:
                    nc.sync.dma_start(
                        tab[r0 : r0 + psizes[pi], :], agin[(li, pi)][:]
                    )
                    return
                nc.gpsimd.collective_compute(
                    "AllGather", mybir.AluOpType.bypass,
                    replica_groups=[list(range(ncores))],
                    ins=[agin[(li, pi)]], outs=[tab[r0:r1, :]],
                )

            copyctr = [0]

            def psum_copy(dst_ap, src_ap, dve_every=4):
                i = copyctr[0]
                copyctr[0] += 1
                if i % dve_every == 0:
                    nc.vector.tensor_copy(dst_ap, src_ap)
                else:
                    nc.scalar.copy(dst_ap, src_ap)

            def layer(l, table_ap, srcW_sb, xT_sb, h1T_out, li, post_t2_hook=None):
                # gather call boundaries: small first calls so the first
                # scatter starts ~one-window-latency after the layer can run
                starts = [0, 1, 4]
                while starts[-1] < NW:
                    starts.append(min(NW, starts[-1] + G))
                while starts[-1] >= NW:
                    starts.pop()
                ncalls = len(starts)
                starts.append(NW)
                w2call = np.searchsorted(starts, np.arange(NW), side="right") - 1
                gtiles = {}

                def issue_gather(ci):
                    if ci in gtiles or ci >= ncalls:
                        return
                    w0 = starts[ci]
                    w = starts[ci + 1] - w0
                    xgw = wp.tile([P, G * H], bf16, name="xgw", tag="xgw", bufs=NXGW)
                    nc.gpsimd.dma_gather(
                        xgw[:, : w * H].rearrange("p (b e) -> p b e", e=H),
                        table_ap,
                        srcW_sb[:, w0 * 8 : (w0 + w) * 8],
                        w * P, w * P, H,
                    )
                    gtiles[ci] = xgw

                def xg(w):
                    ci = int(w2call[w])
                    issue_gather(ci)
                    return gtiles[ci][:, (w - starts[ci]) * H : (w - starts[ci] + 1) * H]

                def prefetch_gathers(gi):
                    # evict calls before this group's first window, then keep
                    # exactly NXGW calls in flight (group + lookahead). Keeps
                    # live tiles <= bufs so pool rotation can never deadlock.
                    wmin = min(ov["w"] for ov in groups[gi])
                    cmin = int(w2call[wmin])
                    for key in [k for k in gtiles if k < cmin]:
                        gtiles.pop(key)
                    for ci in range(cmin, min(cmin + NXGW, ncalls)):
                        issue_gather(ci)

                stiles = {}

                def issue_S(k, span):
                    if k in stiles or k >= NOV:
                        return
                    S = wp.tile([P, SWMAX], bf16, name="S", tag="S", bufs=NSBUF)
                    nc.vector.tensor_scalar(
                        out=S[:, :span], in0=iota_sb[:, :span],
                        scalar1=dqT[:, k : k + 1],
                        scalar2=normT[:, k : k + 1],
                        op0=mybir.AluOpType.is_equal, op1=mybir.AluOpType.mult,
                    )
                    stiles[k] = S

                def issue_group_S(gi):
                    if gi >= len(groups):
                        return
                    for ov in groups[gi]:
                        issue_S(ov["k"], ov["span"])

                pending = None
                first_flush = [True]

                def flush_pending(stop):
                    nonlocal pending
                    if pending is None:
                        return
                    rp_, sbs, msg_ps = pending
                    for s in range(2):
                        r = 2 * rp_ + s
                        for h in range(2):
                            nc.tensor.matmul(
                                msg_ps[h][:],
                                lhsT=wblk_ap(l, r, h),
                                rhs=sbs[h][:, s * T2SZ : (s + 1) * T2SZ],
                                start=first_flush[0] and s == 0,
                                stop=stop and s == 1,
                            )
                    first_flush[0] = False
                    pending = None

                deferred_tail = [None]

                for t2 in range(T2):
                    if t2 == 1 and post_t2_hook is not None:
                        post_t2_hook()
                    msg_ps = {
                        h: pp.tile([P, T2SZ], f32, name=f"m{h}", tag=f"m{h}", bufs=1)
                        for h in range(2)
                    }
                    # layer-1 output stays resident for the next layer's
                    # self-loop; layer-2 output is transient per t2
                    if li == 1:
                        hT_sl = lambda ho, _t2=t2: h1T_out[
                            :, (ho * T2 + _t2) * T2SZ : (ho * T2 + _t2 + 1) * T2SZ
                        ]
                    else:
                        hT_t2 = wp.tile(
                            [P, 2 * T2SZ], f32r, name="h2t", tag="h2t", bufs=2
                        )
                        hT_sl = lambda ho, _t=hT_t2: _t[:, ho * T2SZ : (ho + 1) * T2SZ]
                    first_flush[0] = True
                    for rp in range(RP):
                        gi = t2 * RP + rp
                        prefetch_gathers(gi)
                        issue_group_S(gi)
                        issue_group_S(gi + 1)
                        issue_group_S(gi + 2)
                        agg_ps = {
                            h: pp.tile(
                                [P, 2 * T2SZ], f32, name=f"agg{h}",
                                tag=f"agg{h}", bufs=2,
                            )
                            for h in range(2)
                        }
                        ks = []
                        for h in range(2):
                            for ov in groups[gi]:
                                w, k, fb = ov["w"], ov["k"], ov["fb"]
                                xgt = xg(w)
                                if h == 0:
                                    ks.append(k)
                                S = stiles[k]
                                for (c0s, c1s, st, sp) in ov["parts"]:
                                    nc.tensor.matmul(
                                        agg_ps[h][:, fb * QN + c0s : fb * QN + c1s],
                                        lhsT=xgt[:, h * P : (h + 1) * P],
                                        rhs=S[:, c0s:c1s],
                                        start=st, stop=sp,
                                        skip_group_check=True,
                                    )
                        for k in ks:
                            stiles.pop(k)
                        sbs = {}
                        for h in range(2):
                            aggsb = wp.tile(
                                [P, 2 * T2SZ], f32r, name=f"aggsb{h}",
                                tag=f"aggsb{h}", bufs=3,
                            )
                            psum_copy(aggsb[:], agg_ps[h][:])
                            sbs[h] = aggsb
                        flush_pending(stop=False)
                        pending = (rp, sbs, msg_ps)
                        # previous tile's stores ride behind this tile's
                        # first scatter so its PSUM copies overlap PE work
                        if rp == 0 and deferred_tail[0] is not None:
                            deferred_tail[0]()
                            deferred_tail[0] = None
                    # self-loop emitted before the final flush so it covers
                    # the last group's PSUM->SBUF copy latency
                    for ho in range(2):
                        for h in range(2):
                            nc.tensor.matmul(
                                msg_ps[ho][:],
                                lhsT=loopw_ap(l, h)[:, ho * P : (ho + 1) * P],
                                rhs=xT_sb[:, (h * T2 + t2) * T2SZ : (h * T2 + t2 + 1) * T2SZ],
                                start=False, stop=False,
                            )
                    flush_pending(stop=True)

                    def tail(t2=t2, msg_ps=msg_ps, hT_sl=hT_sl):
                        # msgT -> hT (bias folded into the copy)
                        for ho in range(2):
                            dsl = hT_sl(ho)
                            if ho == 0:
                                nc.scalar.add(
                                    dsl, msg_ps[ho][:],
                                    biasT_sb[:, l * 2 + ho : l * 2 + ho + 1],
                                )
                            else:
                                nc.vector.tensor_scalar(
                                    out=dsl, in0=msg_ps[ho][:],
                                    scalar1=biasT_sb[:, l * 2 + ho : l * 2 + ho + 1],
                                    scalar2=None, op0=mybir.AluOpType.add,
                                )
                        # transpose back to node-major, store to agin piece
                        pi = min(t2 // 3, len(psizes) - 1)
                        t2_0 = pieces_t2[pi][0]
                        for sub in range(2):
                            rows = min(P, NOWN - t2 * T2SZ - sub * P)
                            if rows <= 0:
                                continue
                            tp_ps = pp.tile([P, H], f32r, name="tp", tag="tpx", bufs=1)
                            for ho in range(2):
                                nc.tensor.transpose(
                                    tp_ps[:, ho * P : (ho + 1) * P],
                                    hT_sl(ho)[:, sub * P : (sub + 1) * P],
                                    identr[:],
                                )
                            odt = bf16
                            outb = wp.tile(
                                [P, H], odt, name=f"outb{li}", tag=f"outb{li}", bufs=2
                            )
                            psum_copy(outb[:], tp_ps[:], dve_every=2)
                            o0 = (t2 - t2_0) * T2SZ + sub * P
                            if single:
                                tab = h1tab if li == 1 else h2tab
                                r0 = int(pbase[pi])
                                nc.sync.dma_start(
                                    tab[r0 + o0 : r0 + o0 + rows, :], outb[:rows, :]
                                )
                            else:
                                nc.sync.dma_start(
                                    agin[(li, pi)][o0 : o0 + rows, :], outb[:rows, :]
                                )
                        if not single and t2 == pieces_t2[pi][1] - 1:
                            emit_ag_piece(li, pi)

                    if t2 == T2 - 1:
                        tail()
                    else:
                        deferred_tail[0] = tail

            layer(0, h0b_t.ap()[:], srcW1, x0T, h1T, 1, post_t2_hook=_late_consts)
            for rp in range(R // 2):
                o = ((1 * R + 2 * rp) * 2) * P
                nc.sync.dma_start(
                    wblk_sb[:, o : o + 4 * P], wblk_t.ap()[:, o : o + 4 * P]
                )
            nc.sync.dma_start(loopw_sb[:, 2 * H :], loopw_t.ap()[:, 2 * H :])
            nc.sync.dma_start(drugsW[:], drugsW_t.ap()[:])
            nc.sync.dma_start(targetsW[:], targetsW_t.ap()[:])
            nc.sync.dma_start(fc1_sb[:], fc1_t.ap()[:])
            nc.sync.dma_start(fc1b_sb[:], fc1b_t.ap()[:])
            nc.sync.dma_start(fc2_sb[:], fc2_t.ap()[:])
            layer(1, h1tab[:], srcW2, h1T, None, 2)

            # ---- MLP head, two stages: stage A (pairs [0, HSPLIT)) gathers
            # from the pieces-1..N-1 table slice, so it runs while the last
            # piece is still computing; stage B covers [NP2-B0, NP2) (the
            # overlap with A is recomputed -- identical values, keeps every
            # matmul free dim >= 256 for full-rate f32r).
            NP2 = Q * P
            HS = meta["HSPLIT"]
            pbase3 = meta["pbase3"]
            blo = NP2 - max(2 * P, NP2 - HS)  # stage-B column start
            QA = HS // P
            xdr = wp.tile([P, Q * H], bf16, name="xdr", tag="xdr", bufs=1)
            xtg = wp.tile([P, Q * H], bf16, name="xtg", tag="xtg", bufs=1)
            nc.gpsimd.dma_gather(
                xdr[:, : QA * H].rearrange("p (b e) -> p b e", e=H),
                h2tab[:pbase3, :], drugsW[:, : QA * 8], HS, HS, H,
            )
            nc.gpsimd.dma_gather(
                xtg[:, : QA * H].rearrange("p (b e) -> p b e", e=H),
                h2tab[:pbase3, :], targetsW[:, : QA * 8], HS, HS, H,
            )

            xcatT = [
                wp.tile([P, NP2], f32r, name=f"xcT{k}", tag=f"xcT{k}", bufs=1)
                for k in range(KC)
            ]

            def head_transposes(q_range):
                for k in range(KC):
                    src_sb = xdr if k < KC // 2 else xtg
                    kk = k % (KC // 2)
                    for qq in q_range:
                        ttag = "tpx" if (k * Q + qq) % 2 == 0 else "agg1"
                        tp2_ps = pp.tile(
                            [P, P], bf16, name="tp2h", tag=ttag,
                            bufs=(1 if ttag == "tpx" else 2),
                        )
                        nc.tensor.transpose(
                            tp2_ps[:],
                            src_sb[:, qq * H + kk * P : qq * H + (kk + 1) * P],
                            identb[:],
                        )
                        if qq % 2 == 0:
                            nc.vector.tensor_copy(
                                xcatT[k][:, qq * P : (qq + 1) * P], tp2_ps[:]
                            )
                        else:
                            nc.scalar.copy(
                                xcatT[k][:, qq * P : (qq + 1) * P], tp2_ps[:]
                            )

            z_ps = pp.tile([1, NP2], f32, name="z", tag="hz", bufs=1)
            yTrs = []

            def head_fc(c0, c1, first_stage):
                for m in range(MC):
                    if first_stage:
                        yT_ps = pp.tile([P, NP2], f32, name="yT", tag="agg0", bufs=2)
                        yTr = wp.tile([P, NP2], f32r, name="yTr", tag="yTr", bufs=4)
                        yTrs.append((yT_ps, yTr))
                    else:
                        yT_ps, yTr = yTrs[m]
                    for k in range(KC):
                        nc.tensor.matmul(
                            yT_ps[:, c0:c1],
                            lhsT=fc1_sb[:, (k * MC + m) * P : (k * MC + m + 1) * P],
                            rhs=xcatT[k][:, c0:c1],
                            start=(k == 0), stop=(k == KC - 1),
                            skip_group_check=True,
                        )
                    nc.scalar.activation(
                        yTr[:, c0:c1], yT_ps[:, c0:c1],
                        mybir.ActivationFunctionType.Relu,
                        bias=fc1b_sb[:, m : m + 1], scale=1.0,
                    )
                    nc.tensor.matmul(
                        z_ps[:, c0:c1], lhsT=fc2_sb[:, m : m + 1],
                        rhs=yTr[:, c0:c1],
                        start=(m == 0), stop=(m == MC - 1),
                        skip_group_check=True,
                    )

            head_transposes(range(QA))
            head_fc(0, HS, True)
            # stage B: rest of the pairs, gathered from the full table
            nc.gpsimd.dma_gather(
                xdr[:, QA * H :].rearrange("p (b e) -> p b e", e=H),
                h2tab[:], drugsW[:, QA * 8 :], NP2 - HS, NP2 - HS, H,
            )
            nc.gpsimd.dma_gather(
                xtg[:, QA * H :].rearrange("p (b e) -> p b e", e=H),
                h2tab[:], targetsW[:, QA * 8 :], NP2 - HS, NP2 - HS, H,
            )
            head_transposes(range(QA, Q))
            head_fc(blo, NP2, False)
            zs = wp.tile([1, NP2], f32, name="zs", tag="zs", bufs=1)
            nc.scalar.activation(
                zs[:], z_ps[:], mybir.ActivationFunctionType.Sigmoid,
                bias=meta["fc2b"], scale=1.0,
            )
            nc.sync.dma_start(out_t.ap()[:, :], zs[:])
    return nc


_NC_CACHE = []


def kernel(**inputs):
    from concourse import bass_utils

    meta, in_maps = _preprocess(inputs)
    skey = (meta["N"], meta["H"], meta["R"], meta["NW"], meta["NOV"], meta["Q"],
            str(meta["groups"]))
    if _NC_CACHE and _NC_CACHE[0][0] == skey:
        nc = _NC_CACHE[0][1]
    else:
        nc = _build(meta)
        nc.compile()
        _NC_CACHE[:] = [(skey, nc)]
    res = bass_utils.run_bass_kernel_spmd(nc, in_maps, core_ids=list(range(NCORES)))
    outs = []
    for c in range(NCORES):
        o = np.asarray(res.results[c]["out"])
        inv = np.empty_like(o)
        inv[meta["head_perm"][c]] = o
        outs.append(inv)
    out = np.concatenate(outs, axis=0)
    return out.astype(np.float32)
